# revision 40
# baseline (speedup 1.0000x reference)
"""HGT (heterogeneous graph transformer) layer on 8 trn2 NeuronCores.

Strategy (dst-node 1D sharding, uniform SPMD program):
  - Host folds all small weights:
      WKV[t]    = [W_k[t] | W_v[t]]                      (node-type projections)
      WQA[t,r]  = W_q[t] @ blockdiag(W_att[r] * pri[r,h]/sqrt(dk))
      WMO[r,t]  = blockdiag(W_msg[r]) @ (sigmoid(skip[t])*W_a[t])
    so the per-edge computation needs only RAW k/v of the src node:
      attn[e,h] = qat[rel][dst] . k_raw[src]     (per head, pri pre-folded)
      agg[j]    = sum_r (sum_{e in rel r, dst=j} w_e * v_raw[src]) @ WMO[r]
      out[j]    = agg[j] / s[j]                  (softmax denominator)
  - Each core owns a contiguous range of 6400 dst nodes (single node type).
    Per core the edges are grouped into (node-tile of 128 dst, relation,
    src-half, chunk of 128 edges); chunk structure is the max over cores so
    the SPMD program is identical on all cores, with per-core data padded.
    The src-half split (src < 25600 vs >=) keeps gather indices within
    int16 range for the batched SWDGE dma_gather instruction.
  - All matmul inputs are bf16 (4x PE rate vs fp32); PSUM accumulates fp32.
  - Host pre-transposes h to hT [128, N] bf16, so no PE transposes for the
    projections.  Phase 1 builds the bf16 [N,256] k|v table; a per-core
    qat table [TPC*1024, 128] holds the per-(dst-slot, relation) rotated
    queries.
  - Per node-tile, THREE batched dma_gather ops fetch all edges' k|v rows
    (lo+hi half) and qat rows, spread round-robin over the 4 parallel SWDGE
    queues -- the ~1us descriptor-gen overhead is paid per tile per queue,
    not per 128-edge chunk.
  - Per-edge attn = reduce(qat_g * k_g); segment sums over dst are one-hot
    (edge,dst) matmuls into PSUM accumulated per relation.
  - Padded edge slots get rds=999 -> all-zero one-hot row -> zero
    contribution to both numerator and denominator; their gathered values
    are real (finite) table rows so no NaN risk.
  - Softmax skips the segment-max subtraction: scores are O(1) here, and
    exp(s)/sum(exp(s)) is invariant to the shift.
"""

import os
import sys

sys.path.insert(0, "/opt/trn_rl_repo")

import ml_dtypes
import numpy as np

import concourse.bass as bass
import concourse.bacc as bacc_mod
import concourse.mybir as mybir
import concourse.tile as tile_mod
import concourse.tile_sem_assignment as _tsa
from concourse.bass_utils import run_bass_kernel_spmd
from concourse.masks import make_identity

# The tile framework rotates SWDGE DMAs over the 8 DMASW semaphore lanes in
# scheduled order, but each physical semaphore is locked to one SWDGE queue.
# To run gathers on all 4 queues concurrently, pin each queue to its own pair
# of lanes so a semaphore only ever sees one queue.
_ORIG_ASSIGN_TICK = _tsa.TileClockTick._assign_tick


def _qlane_assign_tick(self, inst):
    qnum = getattr(inst, "queue_num", None)
    if (qnum is not None and inst.engine == mybir.EngineType.Pool
            and isinstance(inst, _tsa.DMAInst)
            and not isinstance(inst, _tsa.bass_isa.UserSyncedRemoteDMADescs)
            and self.swdge_sem_count >= 8):
        cnt = getattr(self, "_qlane_cnt", None)
        if cnt is None:
            cnt = self._qlane_cnt = [0, 0, 0, 0]
        lanes = self.swdge_sem_count // 4
        save = self.next_sw_dma_idx
        self.next_sw_dma_idx = qnum * lanes + (cnt[qnum] % lanes)
        cnt[qnum] += 1
        try:
            return _ORIG_ASSIGN_TICK(self, inst)
        finally:
            self.next_sw_dma_idx = save
    return _ORIG_ASSIGN_TICK(self, inst)


_tsa.TileClockTick._assign_tick = _qlane_assign_tick

F32 = mybir.dt.float32
BF16 = mybir.dt.bfloat16
F8 = mybir.dt.float8e4
I16 = mybir.dt.int16
NPBF16 = ml_dtypes.bfloat16
NPF8 = ml_dtypes.float8_e4m3

N, E, T, R, NH, DK, D = 51200, 640000, 4, 8, 4, 32, 128
NCORES = 8
NPC = N // NCORES          # 6400 nodes per core
TPC = NPC // 128           # 50 node-tiles per core
TT = N // 128              # 400 table tiles
NPT = N // T               # nodes per type
EPR = E // R               # edges per relation
NHALF = N // 2             # src-half split for int16 gather indices
SQRT_DK = float(np.sqrt(DK))


def _blockdiag(W):
    """[R,H,dk,dk] -> [R,D,D] block-diagonal per head."""
    out = np.zeros((R, D, D), np.float32)
    for r in range(R):
        for hh in range(NH):
            out[r, hh * DK:(hh + 1) * DK, hh * DK:(hh + 1) * DK] = W[r, hh]
    return out


def _wrap16(L):
    """Linear int16 index list -> [128, ceil(n/16)] SBUF layout (wrapped in 16
    partitions, replicated to all 8 16-partition groups)."""
    n = L.shape[0]
    F = -(-n // 16)
    pad = np.zeros(F * 16, np.int16)
    pad[:n] = L
    seg = pad.reshape(F, 16).T               # [16, F]
    return np.tile(seg, (8, 1))              # [128, F]


def _host_prep(h, k_linears, q_linears, v_linears, a_linears,
               relation_att, relation_msg, relation_pri, skip,
               row_idx, col_idx):
    pri = np.asarray(relation_pri, np.float32) / SQRT_DK               # [R,H]
    att = np.asarray(relation_att, np.float32) * pri[:, :, None, None]
    Watt = _blockdiag(att)
    Wmsg = _blockdiag(np.asarray(relation_msg, np.float32))
    skip = np.asarray(skip, np.float32)
    Wout = (1.0 / (1.0 + np.exp(-skip))).astype(np.float32) * np.asarray(a_linears, np.float32)
    WQA = np.einsum("tab,rbc->trac", np.asarray(q_linears, np.float32), Watt)
    WMO = np.einsum("rab,tbc->rtac", Wmsg, Wout)
    WKV = np.concatenate([np.asarray(k_linears, np.float32),
                          np.asarray(v_linears, np.float32)], axis=2)  # [T,D,256]

    row = np.asarray(row_idx, np.int64)
    col = np.asarray(col_idx, np.int64)
    erel = np.arange(E, dtype=np.int64) // EPR
    half = (row >= NHALF).astype(np.int64)

    core = col // NPC
    tl = (col % NPC) // 128
    # per-(core,tile,rel,half) edge counts
    key = ((core * TPC + tl) * R + erel) * 2 + half
    counts = np.bincount(key, minlength=NCORES * TPC * R * 2).reshape(NCORES, TPC, R, 2)
    maxcnt = counts.max(axis=0)                                       # [TPC,R,2]
    n_chunks = -(-maxcnt // 128)                                      # ceil
    # ensure at least one chunk per tile (degenerate safety)
    C_lo_t = n_chunks[:, :, 0].sum(axis=1)
    C_hi_t = n_chunks[:, :, 1].sum(axis=1)
    # chunk index base per (tile, rel, half): lo chunks first, then hi
    chunk_base = np.zeros((TPC, R, 2), np.int64)
    for t in range(TPC):
        off = 0
        for r in range(R):
            chunk_base[t, r, 0] = off
            off += n_chunks[t, r, 0]
        for r in range(R):
            chunk_base[t, r, 1] = off
            off += n_chunks[t, r, 1]
    C_t = C_lo_t + C_hi_t
    Cmax = int(C_t.max())

    # per-core padded metadata arrays
    idx_all = np.zeros((NCORES, TPC, 128, Cmax), np.int16)   # kv idx (half-local)
    idx2_all = np.zeros((NCORES, TPC, 128, Cmax), np.int16)  # qat idx (tile-local)
    rds_all = np.full((NCORES, TPC, 128, Cmax), 999.0, NPBF16)

    order = np.argsort(key, kind="stable")
    ranks = np.empty(E, np.int64)
    group_start = np.zeros(NCORES * TPC * R * 2, np.int64)
    cnt_flat = counts.reshape(-1)
    np.cumsum(cnt_flat[:-1], out=group_start[1:])
    ranks[order] = np.arange(E) - group_start[key[order]]

    chunk_of = chunk_base[tl, erel, half] + ranks // 128              # [E]
    part_of = ranks % 128
    rd = col % 128
    idx_all[core, tl, part_of, chunk_of] = (row - half * NHALF).astype(np.int16)
    idx2_all[core, tl, part_of, chunk_of] = (erel * 128 + rd).astype(np.int16)
    rds_all[core, tl, part_of, chunk_of] = rd.astype(NPBF16)

    # chunk -> relation map per tile (lo section then hi section)
    chunk_rel = []
    for t in range(TPC):
        rels = []
        for hh in range(2):
            for r in range(R):
                rels += [r] * int(n_chunks[t, r, hh])
        chunk_rel.append(rels)

    # int16 gather index lists, wrap16 layout, concatenated per core:
    # per tile: [kv-lo (C_lo*8) | kv-hi (C_hi*8)] columns
    idx16 = np.zeros((NCORES, 128, 8 * int(C_t.sum())), np.int16)
    seg_off = []          # per tile: (lo_off, hi_off) in i16 columns
    off = 0
    for t in range(TPC):
        clo, chi, ct = int(C_lo_t[t]), int(C_hi_t[t]), int(C_t[t])
        seg_off.append((off, off + clo * 8))
        for c in range(NCORES):
            g = idx_all[c, t, :, :ct]          # [128, C]
            lo = _wrap16(np.ascontiguousarray(g[:, :clo].T).reshape(-1))
            hi = _wrap16(np.ascontiguousarray(g[:, clo:ct].T).reshape(-1))
            idx16[c, :, off:off + clo * 8] = lo
            idx16[c, :, off + clo * 8:off + ct * 8] = hi
        off += ct * 8
    idx16 = np.ascontiguousarray(idx16[:, :, :off])
    TOT16 = off

    # one-hot O[e, (c, j)] and O^T[j, (c, e)] shipped directly as fp8
    # (0/1 exact); padded slots (rds=999) give all-zero rows/cols
    jj = np.arange(128)
    o_all = np.zeros((NCORES, TPC, 128, Cmax * 128), NPF8)
    ot_all = np.zeros((NCORES, TPC, 128, Cmax * 128), NPF8)
    rds_f = rds_all.astype(np.float32)
    for c in range(NCORES):
        for t in range(TPC):
            oh = (rds_f[c, t][:, :, None] == jj[None, None, :])   # [e, C, j]
            o_all[c, t] = oh.reshape(128, -1).astype(NPF8)
            ot_all[c, t] = oh.transpose(2, 1, 0).reshape(128, -1).astype(NPF8)

    hT = np.ascontiguousarray(np.asarray(h, np.float32).T.astype(NPBF16))  # [128, N]
    hexp = np.zeros((NH, D), np.float32)      # head expander: hexp[h, d] = (d//DK == h)
    for hh in range(NH):
        hexp[hh, hh * DK:(hh + 1) * DK] = 1.0


    in_maps = []
    for c in range(NCORES):
        t_c = (c * NPC) // NPT
        in_maps.append({
            "ht": hT,
            "ownht": np.ascontiguousarray(hT[:, c * NPC:(c + 1) * NPC]),
            "wkv": np.ascontiguousarray(
                WKV.transpose(1, 0, 2).reshape(D, T * 256).astype(NPBF16)),
            "wqa": np.ascontiguousarray(
                WQA[t_c].transpose(1, 0, 2).reshape(D, R * D).astype(NPBF16)),
            "wmo": np.ascontiguousarray(
                WMO[:, t_c].transpose(1, 0, 2).reshape(D, R * D).astype(NPBF16)),
            "idx16": idx16[c],
            "oall": o_all[c],
            "otall": ot_all[c],
            "hexp": hexp,
        })
    meta = dict(chunk_rel=chunk_rel, C_lo=C_lo_t, C_hi=C_hi_t, C_t=C_t,
                Cmax=Cmax, seg_off=seg_off, TOT16=TOT16)
    return in_maps, meta


def _build_program(meta):
    chunk_rel, C_lo, C_hi, C_t = (meta["chunk_rel"], meta["C_lo"],
                                  meta["C_hi"], meta["C_t"])
    Cmax, seg_off, TOT16 = meta["Cmax"], meta["seg_off"], meta["TOT16"]

    nc = bacc_mod.Bacc(num_swdge_queues=4)
    ht_ext = nc.declare_dram_parameter("ht", [D, N], BF16, isOutput=False)
    ownht_ext = nc.declare_dram_parameter("ownht", [D, NPC], BF16, isOutput=False)
    wkv_ext = nc.declare_dram_parameter("wkv", [D, T * 256], BF16, isOutput=False)
    wqa_ext = nc.declare_dram_parameter("wqa", [D, R * D], BF16, isOutput=False)
    wmo_ext = nc.declare_dram_parameter("wmo", [D, R * D], BF16, isOutput=False)
    idx16_ext = nc.declare_dram_parameter("idx16", [128, TOT16], I16, isOutput=False)
    oall_ext = nc.declare_dram_parameter("oall", [TPC, 128, Cmax * 128], F8, isOutput=False)
    otall_ext = nc.declare_dram_parameter("otall", [TPC, 128, Cmax * 128], F8, isOutput=False)
    hexp_ext = nc.declare_dram_parameter("hexp", [NH, D], F32, isOutput=False)
    out_ext = nc.declare_dram_parameter("out", [NPC, D], F32, isOutput=True)

    kv_dram = nc.dram_tensor("kv_table", [N, 2 * D], BF16)

    NB = TT // 8               # phase-1 batches of 8 tiles (1024 nodes)
    Exp = mybir.ActivationFunctionType.Exp
    Copy = mybir.ActivationFunctionType.Copy

    with tile_mod.TileContext(nc) as tc:
        with (
            tc.tile_pool(name="const", bufs=1) as cp,
            tc.tile_pool(name="ph1", bufs=3) as p1,
            tc.tile_pool(name="qatp", bufs=4) as qp,
            tc.tile_pool(name="tile", bufs=3) as tp,
            tc.tile_pool(name="ps_half", bufs=1, space="PSUM") as ps_half,
            tc.tile_pool(name="ps_qep", bufs=2, space="PSUM") as ps_qep,
            tc.tile_pool(name="ps_at", bufs=2, space="PSUM") as ps_at,
            tc.tile_pool(name="ps_sm", bufs=1, space="PSUM") as ps_sm,
        ):
            hexp_sb = cp.tile([NH, D], F32)
            nc.sync.dma_start(out=hexp_sb[:], in_=hexp_ext[:])
            ident = cp.tile([128, 128], F32)
            make_identity(nc, ident[:])
            wkv_sb = cp.tile([128, T * 256], BF16)
            nc.sync.dma_start(out=wkv_sb[:], in_=wkv_ext[:])
            wqa_sb = cp.tile([128, R * D], BF16)
            nc.sync.dma_start(out=wqa_sb[:], in_=wqa_ext[:])
            wmo_sb = cp.tile([128, R * D], BF16)
            nc.sync.dma_start(out=wmo_sb[:], in_=wmo_ext[:])
            ownht = cp.tile([128, NPC], BF16)
            nc.sync.dma_start(out=ownht[:], in_=ownht_ext[:])
            idx16_sb = cp.tile([128, TOT16], I16)
            nc.sync.dma_start(out=idx16_sb[:], in_=idx16_ext[:])

            # ---- phase 1: bf16 k|v table for all N nodes ----
            for b in range(NB):
                htc = p1.tile([128, 1024], BF16, tag="htc")
                nc.sync.dma_start(out=htc[:], in_=ht_ext[:, b * 1024:(b + 1) * 1024])
                kvs = p1.tile([128, 2048], BF16, tag="kvs")
                for i in range(4):
                    ty = (8 * b + 2 * i) // (NPT // 128)
                    kvp = ps_half.tile([128, 512], F32, tag="half")
                    for j in range(2):
                        nc.tensor.matmul(kvp[:, j * 256:(j + 1) * 256],
                                         lhsT=htc[:, (2 * i + j) * 128:(2 * i + j + 1) * 128],
                                         rhs=wkv_sb[:, ty * 256:(ty + 1) * 256],
                                         start=True, stop=True)
                    nc.vector.tensor_copy(kvs[:, i * 512:(i + 1) * 512], kvp[:])
                nc.sync.dma_start(
                    out=kv_dram[b * 1024:(b + 1) * 1024, :]
                        .rearrange("(t p) k -> p t k", p=128),
                    in_=kvs[:].rearrange("p (t k) -> p t k", t=8))

            # ---- phase 2: per node-tile edge processing ----
            qn = 0
            for tl in range(TPC):
                C = int(C_t[tl])
                CL = int(C_lo[tl])
                CH = int(C_hi[tl])
                rels = chunk_rel[tl]
                lo_off, hi_off = seg_off[tl]

                # per-tile rotated queries (stay in SBUF)
                qat = qp.tile([128, R * D], BF16, tag="qat")
                for i in range(2):
                    qah = ps_half.tile([128, 512], F32, tag="half")
                    nc.tensor.matmul(qah[:],
                                     lhsT=ownht[:, tl * 128:(tl + 1) * 128],
                                     rhs=wqa_sb[:, i * 512:(i + 1) * 512],
                                     start=True, stop=True)
                    nc.scalar.activation(out=qat[:, i * 512:(i + 1) * 512],
                                         in_=qah[:], func=Copy)

                # single_packet dma_gather caps at 64 descs/lane = 1024 idxs
                # = 8 chunks per op; split larger gathers into 8-chunk spans
                def gather_spans(dst, dst_col0, src_ap, idx_col0, nch, es):
                    nonlocal qn
                    for s0 in range(0, nch, 8):
                        sc = min(8, nch - s0)
                        nc.gpsimd.dma_gather(
                            dst[:, (dst_col0 + s0) * es:(dst_col0 + s0 + sc) * es]
                                .rearrange("p (c x) -> p c x", x=es),
                            src_ap,
                            idx16_sb[:, idx_col0 + s0 * 8:idx_col0 + (s0 + sc) * 8],
                            sc * 128, sc * 128, es, queue_num=qn % 4)
                        qn += 1

                kvg = tp.tile([128, Cmax * 256], BF16, tag="kvg")
                if CL:
                    gather_spans(kvg, 0, kv_dram[0:NHALF, :], lo_off, CL, 256)
                if CH:
                    gather_spans(kvg, CL, kv_dram[NHALF:N, :], hi_off, CH, 256)

                # one-hot O[e, (c,j)] and O^T[j, (c,e)] DMAd from host (fp8)
                Oall = tp.tile([128, Cmax * 128], F8, tag="Oall")
                nc.sync.dma_start(out=Oall[:, :C * 128], in_=oall_ext[tl, :, :C * 128])
                OT = tp.tile([128, Cmax * 128], F8, tag="OT")
                nc.sync.dma_start(out=OT[:, :C * 128], in_=otall_ext[tl, :, :C * 128])

                # qep[e, d] = qat[dst_e, rel_e, d] via one-hot matmuls, in
                # PSUM waves of 8 chunks; then attn = sum_d qep * k per head
                prod = tp.tile([128, Cmax * 128], BF16, tag="prod")
                for w0 in range(0, C, 4):
                    nw = min(4, C - w0)
                    qepw = ps_qep.tile([128, 512], F32, tag="qep")
                    for c in range(w0, w0 + nw):
                        rc = rels[c]
                        nc.tensor.matmul(qepw[:, (c - w0) * 128:(c - w0 + 1) * 128],
                                         lhsT=OT[:, c * 128:(c + 1) * 128],
                                         rhs=qat[:, rc * 128:(rc + 1) * 128],
                                         start=True, stop=True)
                    nc.vector.tensor_tensor(
                        out=prod[:, w0 * 128:(w0 + nw) * 128]
                            .rearrange("p (c d) -> p c d", c=nw),
                        in0=qepw[:, :nw * 128].rearrange("p (c d) -> p c d", c=nw),
                        in1=kvg[:, :C * 256].rearrange("p (c x) -> p c x", c=C)[:, w0:w0 + nw, 0:128],
                        op=mybir.AluOpType.mult,
                    )

                # pairwise tree-reduce of the 32 dk dims per head
                cur, n = prod, C * 128
                for lvl in range(4):
                    nxt = tp.tile([128, Cmax * (64 >> lvl)], BF16, tag=f"red{lvl}")
                    v = cur[:, :n].rearrange("p (g t) -> p g t", t=2)
                    nc.vector.tensor_tensor(
                        out=nxt[:, :n // 2].rearrange("p (g t) -> p g t", t=1),
                        in0=v[:, :, 0:1], in1=v[:, :, 1:2],
                        op=mybir.AluOpType.add,
                    )
                    cur, n = nxt, n // 2
                attn = tp.tile([128, Cmax * NH], F32, tag="attn")
                v = cur[:, :n].rearrange("p (g t) -> p g t", t=2)
                nc.vector.tensor_tensor(
                    out=attn[:, :C * NH].rearrange("p (g t) -> p g t", t=1),
                    in0=v[:, :, 0:1], in1=v[:, :, 1:2],
                    op=mybir.AluOpType.add,
                )
                wv = tp.tile([128, Cmax * NH], BF16, tag="wv")
                nc.scalar.activation(out=wv[:, :C * NH], in_=attn[:, :C * NH], func=Exp)

                # wm[e, d] = w[e, h(d)] * v_raw[src_e, d]
                wmt = tp.tile([128, Cmax * 128], BF16, tag="wmt")
                nc.vector.tensor_tensor(
                    out=wmt[:, :C * 128].rearrange("p (c h d) -> p c h d", c=C, h=NH),
                    in0=kvg[:, :C * 256].rearrange("p (c x) -> p c x", c=C)[:, :, 128:256]
                        .rearrange("p c (h d) -> p c h d", h=NH),
                    in1=wv[:, :C * NH].rearrange("p (c h u) -> p c h u", c=C, u=1)
                        .to_broadcast([128, C, NH, DK]),
                    op=mybir.AluOpType.mult,
                )

                # segment sums into PSUM: A_T[d, j] per relation block + s[j, h]
                # PSUM start=True marks the whole 2KB zero region pending --
                # accumulation groups sharing a bank must run back-to-back,
                # so iterate chunks grouped by relation (data layout unchanged)
                ATp = ps_at.tile([128, R * D], F32, tag="ATp")
                sp = ps_sm.tile([128, 128], F32, tag="sm")
                order = sorted(range(C), key=lambda c: rels[c])
                for k, c in enumerate(order):
                    rc = rels[c]
                    first = (k == 0) or rels[order[k - 1]] != rc
                    last = (k == C - 1) or rels[order[k + 1]] != rc
                    nc.tensor.matmul(ATp[:, rc * D:(rc + 1) * D],
                                     lhsT=wmt[:, c * 128:(c + 1) * 128],
                                     rhs=Oall[:, c * 128:(c + 1) * 128],
                                     start=first, stop=last, skip_group_check=True)
                for c in range(C):
                    nc.tensor.matmul(sp[:, :NH], lhsT=Oall[:, c * 128:(c + 1) * 128],
                                     rhs=wv[:, c * NH:(c + 1) * NH],
                                     start=(c == 0), stop=(c == C - 1),
                                     skip_group_check=True)

                rec = tp.tile([128, NH], F32, tag="rec")
                nc.vector.reciprocal(rec[:], sp[:, :NH])
                # rts[d, j] = rec[j, h(d)] via tiny transpose + K=4 matmul
                # against the constant head-expander hexp[h, d] = (h(d) == h)
                rtp = ps_sm.tile([128, 128], F32, tag="sm")
                nc.tensor.transpose(rtp[:NH, :], rec[:], ident[:])
                recT = tp.tile([NH, 128], F32, tag="recT")
                nc.scalar.activation(out=recT[:], in_=rtp[:NH, :], func=Copy)
                rts2 = ps_sm.tile([128, 128], F32, tag="sm")
                nc.tensor.matmul(rts2[:], lhsT=hexp_sb[:], rhs=recT[:],
                                 start=True, stop=True)
                rts = tp.tile([128, 128], F32, tag="rts")
                nc.scalar.activation(out=rts[:], in_=rts2[:], func=Copy)

                Anorm = tp.tile([128, R * D], BF16, tag="Anorm")
                nc.vector.tensor_tensor(
                    out=Anorm[:].rearrange("p (r j) -> p r j", r=R),
                    in0=ATp[:].rearrange("p (r j) -> p r j", r=R),
                    in1=rts[:].rearrange("p (u j) -> p u j", u=1).to_broadcast([128, R, 128]),
                    op=mybir.AluOpType.mult,
                )

                outp = ps_sm.tile([128, 128], F32, tag="sm")
                for r in range(R):
                    nc.tensor.matmul(outp[:], lhsT=Anorm[:, r * D:(r + 1) * D],
                                     rhs=wmo_sb[:, r * D:(r + 1) * D],
                                     start=(r == 0), stop=(r == R - 1))
                osb = tp.tile([128, 128], F32, tag="osb")
                nc.scalar.activation(out=osb[:], in_=outp[:], func=Copy)
                nc.sync.dma_start(out=out_ext[tl * 128:(tl + 1) * 128, :], in_=osb[:])
    nc.compile()
    return nc


def kernel(h, k_linears, q_linears, v_linears, a_linears,
           relation_att, relation_msg, relation_pri, skip,
           row_idx, col_idx, eids, **_unused):
    in_maps, meta = _host_prep(
        h, k_linears, q_linears, v_linears, a_linears,
        relation_att, relation_msg, relation_pri, skip, row_idx, col_idx)
    nc = _build_program(meta)
    kw = {}
    if os.environ.get("KBENCH_TRACE"):
        kw = dict(trace=True, tmpdir=os.environ.get("KBENCH_TMPDIR") or None)
    res = run_bass_kernel_spmd(nc, in_maps, list(range(NCORES)), **kw)
    global LAST_RESULTS
    LAST_RESULTS = res
    out = np.concatenate([res.results[c]["out"] for c in range(NCORES)], axis=0)
    return out.astype(np.float32)


LAST_RESULTS = None


# revision 41
# speedup vs baseline: 1.1966x; 1.1966x over previous
"""HGT (heterogeneous graph transformer) layer on 8 trn2 NeuronCores.

Strategy (dst-node 1D sharding, uniform SPMD program):
  - Host folds all small weights:
      WKV[t]    = [W_k[t] | W_v[t]]                      (node-type projections)
      WQA[t,r]  = W_q[t] @ blockdiag(W_att[r] * pri[r,h]/sqrt(dk))
      WMO[r,t]  = blockdiag(W_msg[r]) @ (sigmoid(skip[t])*W_a[t])
    so the per-edge computation needs only RAW k/v of the src node:
      attn[e,h] = qat[rel][dst] . k_raw[src]     (per head, pri pre-folded)
      agg[j]    = sum_r (sum_{e in rel r, dst=j} w_e * v_raw[src]) @ WMO[r]
      out[j]    = agg[j] / s[j]                  (softmax denominator)
  - Each core owns a contiguous range of 6400 dst nodes (single node type).
    Per core the edges are grouped into (node-tile of 128 dst, relation,
    src-half, chunk of 128 edges); chunk structure is the max over cores so
    the SPMD program is identical on all cores, with per-core data padded.
    The src-half split (src < 25600 vs >=) keeps gather indices within
    int16 range for the batched SWDGE dma_gather instruction.
  - All matmul inputs are bf16 (4x PE rate vs fp32); PSUM accumulates fp32.
  - Host pre-transposes h to hT [128, N] bf16, so no PE transposes for the
    projections.  Phase 1 builds the bf16 [N,256] k|v table; a per-core
    qat table [TPC*1024, 128] holds the per-(dst-slot, relation) rotated
    queries.
  - Per node-tile, THREE batched dma_gather ops fetch all edges' k|v rows
    (lo+hi half) and qat rows, spread round-robin over the 4 parallel SWDGE
    queues -- the ~1us descriptor-gen overhead is paid per tile per queue,
    not per 128-edge chunk.
  - Per-edge attn = reduce(qat_g * k_g); segment sums over dst are one-hot
    (edge,dst) matmuls into PSUM accumulated per relation.
  - Padded edge slots get rds=999 -> all-zero one-hot row -> zero
    contribution to both numerator and denominator; their gathered values
    are real (finite) table rows so no NaN risk.
  - Softmax skips the segment-max subtraction: scores are O(1) here, and
    exp(s)/sum(exp(s)) is invariant to the shift.
"""

import os
import sys

sys.path.insert(0, "/opt/trn_rl_repo")

import ml_dtypes
import numpy as np

import concourse.bass as bass
import concourse.bacc as bacc_mod
import concourse.mybir as mybir
import concourse.tile as tile_mod
import concourse.tile_sem_assignment as _tsa
from concourse.bass_utils import run_bass_kernel_spmd
from concourse.masks import make_identity

# The tile framework rotates SWDGE DMAs over the 8 DMASW semaphore lanes in
# scheduled order, but each physical semaphore is locked to one SWDGE queue.
# To run gathers on all 4 queues concurrently, pin each queue to its own pair
# of lanes so a semaphore only ever sees one queue.
_ORIG_ASSIGN_TICK = _tsa.TileClockTick._assign_tick


def _qlane_assign_tick(self, inst):
    qnum = getattr(inst, "queue_num", None)
    if (qnum is not None and inst.engine == mybir.EngineType.Pool
            and isinstance(inst, _tsa.DMAInst)
            and not isinstance(inst, _tsa.bass_isa.UserSyncedRemoteDMADescs)
            and self.swdge_sem_count >= 8):
        cnt = getattr(self, "_qlane_cnt", None)
        if cnt is None:
            cnt = self._qlane_cnt = [0, 0, 0, 0]
        lanes = self.swdge_sem_count // 4
        save = self.next_sw_dma_idx
        self.next_sw_dma_idx = qnum * lanes + (cnt[qnum] % lanes)
        cnt[qnum] += 1
        try:
            return _ORIG_ASSIGN_TICK(self, inst)
        finally:
            self.next_sw_dma_idx = save
    return _ORIG_ASSIGN_TICK(self, inst)


_tsa.TileClockTick._assign_tick = _qlane_assign_tick

F32 = mybir.dt.float32
BF16 = mybir.dt.bfloat16
F8 = mybir.dt.float8e4
I16 = mybir.dt.int16
NPBF16 = ml_dtypes.bfloat16
NPF8 = ml_dtypes.float8_e4m3

N, E, T, R, NH, DK, D = 51200, 640000, 4, 8, 4, 32, 128
NCORES = 8
NPC = N // NCORES          # 6400 nodes per core
TPC = NPC // 128           # 50 node-tiles per core
TT = N // 128              # 400 table tiles
NPT = N // T               # nodes per type
EPR = E // R               # edges per relation
NHALF = N // 2             # src-half split for int16 gather indices
SQRT_DK = float(np.sqrt(DK))


def _blockdiag(W):
    """[R,H,dk,dk] -> [R,D,D] block-diagonal per head."""
    out = np.zeros((R, D, D), np.float32)
    for r in range(R):
        for hh in range(NH):
            out[r, hh * DK:(hh + 1) * DK, hh * DK:(hh + 1) * DK] = W[r, hh]
    return out


def _wrap16(L):
    """Linear int16 index list -> [128, ceil(n/16)] SBUF layout (wrapped in 16
    partitions, replicated to all 8 16-partition groups)."""
    n = L.shape[0]
    F = -(-n // 16)
    pad = np.zeros(F * 16, np.int16)
    pad[:n] = L
    seg = pad.reshape(F, 16).T               # [16, F]
    return np.tile(seg, (8, 1))              # [128, F]


def _host_prep(h, k_linears, q_linears, v_linears, a_linears,
               relation_att, relation_msg, relation_pri, skip,
               row_idx, col_idx):
    pri = np.asarray(relation_pri, np.float32) / SQRT_DK               # [R,H]
    att = np.asarray(relation_att, np.float32) * pri[:, :, None, None]
    Watt = _blockdiag(att)
    Wmsg = _blockdiag(np.asarray(relation_msg, np.float32))
    skip = np.asarray(skip, np.float32)
    Wout = (1.0 / (1.0 + np.exp(-skip))).astype(np.float32) * np.asarray(a_linears, np.float32)
    WQA = np.einsum("tab,rbc->trac", np.asarray(q_linears, np.float32), Watt)
    WMO = np.einsum("rab,tbc->rtac", Wmsg, Wout)
    WKV = np.concatenate([np.asarray(k_linears, np.float32),
                          np.asarray(v_linears, np.float32)], axis=2)  # [T,D,256]

    row = np.asarray(row_idx, np.int64)
    col = np.asarray(col_idx, np.int64)
    erel = np.arange(E, dtype=np.int64) // EPR
    half = (row >= NHALF).astype(np.int64)

    core = col // NPC
    tl = (col % NPC) // 128
    # per-(core,tile,rel,half) edge counts
    key = ((core * TPC + tl) * R + erel) * 2 + half
    counts = np.bincount(key, minlength=NCORES * TPC * R * 2).reshape(NCORES, TPC, R, 2)
    maxcnt = counts.max(axis=0)                                       # [TPC,R,2]
    n_chunks = -(-maxcnt // 128)                                      # ceil
    # ensure at least one chunk per tile (degenerate safety)
    C_lo_t = n_chunks[:, :, 0].sum(axis=1)
    C_hi_t = n_chunks[:, :, 1].sum(axis=1)
    # chunk index base per (tile, rel, half): lo chunks first, then hi
    chunk_base = np.zeros((TPC, R, 2), np.int64)
    for t in range(TPC):
        off = 0
        for r in range(R):
            chunk_base[t, r, 0] = off
            off += n_chunks[t, r, 0]
        for r in range(R):
            chunk_base[t, r, 1] = off
            off += n_chunks[t, r, 1]
    C_t = C_lo_t + C_hi_t
    Cmax = int(C_t.max())

    # per-core padded metadata arrays
    idx_all = np.zeros((NCORES, TPC, 128, Cmax), np.int16)   # kv idx (half-local)
    idx2_all = np.zeros((NCORES, TPC, 128, Cmax), np.int16)  # qat idx (tile-local)
    rds_all = np.full((NCORES, TPC, 128, Cmax), 999.0, NPBF16)

    order = np.argsort(key, kind="stable")
    ranks = np.empty(E, np.int64)
    group_start = np.zeros(NCORES * TPC * R * 2, np.int64)
    cnt_flat = counts.reshape(-1)
    np.cumsum(cnt_flat[:-1], out=group_start[1:])
    ranks[order] = np.arange(E) - group_start[key[order]]

    chunk_of = chunk_base[tl, erel, half] + ranks // 128              # [E]
    part_of = ranks % 128
    rd = col % 128
    idx_all[core, tl, part_of, chunk_of] = (row - half * NHALF).astype(np.int16)
    idx2_all[core, tl, part_of, chunk_of] = (erel * 128 + rd).astype(np.int16)
    rds_all[core, tl, part_of, chunk_of] = rd.astype(NPBF16)

    # chunk -> relation map per tile (lo section then hi section)
    chunk_rel = []
    for t in range(TPC):
        rels = []
        for hh in range(2):
            for r in range(R):
                rels += [r] * int(n_chunks[t, r, hh])
        chunk_rel.append(rels)

    # int16 gather index lists, wrap16 layout, concatenated per core:
    # per tile: [kv-lo (C_lo*8) | kv-hi (C_hi*8)] columns
    idx16 = np.zeros((NCORES, 128, 8 * int(C_t.sum())), np.int16)
    seg_off = []          # per tile: (lo_off, hi_off) in i16 columns
    off = 0
    for t in range(TPC):
        clo, chi, ct = int(C_lo_t[t]), int(C_hi_t[t]), int(C_t[t])
        seg_off.append((off, off + clo * 8))
        for c in range(NCORES):
            g = idx_all[c, t, :, :ct]          # [128, C]
            lo = _wrap16(np.ascontiguousarray(g[:, :clo].T).reshape(-1))
            hi = _wrap16(np.ascontiguousarray(g[:, clo:ct].T).reshape(-1))
            idx16[c, :, off:off + clo * 8] = lo
            idx16[c, :, off + clo * 8:off + ct * 8] = hi
        off += ct * 8
    idx16 = np.ascontiguousarray(idx16[:, :, :off])
    TOT16 = off

    # one-hot O[e, (c, j)] and O^T[j, (c, e)] shipped directly as fp8
    # (0/1 exact); padded slots (rds=999) give all-zero rows/cols
    jj = np.arange(128)
    o_all = np.zeros((NCORES, TPC, 128, Cmax * 128), NPF8)
    ot_all = np.zeros((NCORES, TPC, 128, Cmax * 128), NPF8)
    rds_f = rds_all.astype(np.float32)
    for c in range(NCORES):
        for t in range(TPC):
            oh = (rds_f[c, t][:, :, None] == jj[None, None, :])   # [e, C, j]
            o_all[c, t] = oh.reshape(128, -1).astype(NPF8)
            ot_all[c, t] = oh.transpose(2, 1, 0).reshape(128, -1).astype(NPF8)

    hT = np.ascontiguousarray(np.asarray(h, np.float32).T.astype(NPBF16))  # [128, N]
    hexp = np.zeros((NH, D), np.float32)      # head expander: hexp[h, d] = (d//DK == h)
    for hh in range(NH):
        hexp[hh, hh * DK:(hh + 1) * DK] = 1.0


    in_maps = []
    for c in range(NCORES):
        t_c = (c * NPC) // NPT
        in_maps.append({
            "ht": hT,
            "ownht": np.ascontiguousarray(hT[:, c * NPC:(c + 1) * NPC]),
            "wkv": np.ascontiguousarray(
                WKV.transpose(1, 0, 2).reshape(D, T * 256).astype(NPBF16)),
            "wqa": np.ascontiguousarray(
                WQA[t_c].transpose(1, 0, 2).reshape(D, R * D).astype(NPBF16)),
            "wmo": np.ascontiguousarray(
                WMO[:, t_c].transpose(1, 0, 2).reshape(D, R * D).astype(NPBF16)),
            "idx16": idx16[c],
            "oall": o_all[c],
            "otall": ot_all[c],
            "hexp": hexp,
        })
    meta = dict(chunk_rel=chunk_rel, C_lo=C_lo_t, C_hi=C_hi_t, C_t=C_t,
                Cmax=Cmax, seg_off=seg_off, TOT16=TOT16)
    return in_maps, meta


def _build_program(meta):
    chunk_rel, C_lo, C_hi, C_t = (meta["chunk_rel"], meta["C_lo"],
                                  meta["C_hi"], meta["C_t"])
    Cmax, seg_off, TOT16 = meta["Cmax"], meta["seg_off"], meta["TOT16"]

    nc = bacc_mod.Bacc(num_swdge_queues=4)
    ht_ext = nc.declare_dram_parameter("ht", [D, N], BF16, isOutput=False)
    ownht_ext = nc.declare_dram_parameter("ownht", [D, NPC], BF16, isOutput=False)
    wkv_ext = nc.declare_dram_parameter("wkv", [D, T * 256], BF16, isOutput=False)
    wqa_ext = nc.declare_dram_parameter("wqa", [D, R * D], BF16, isOutput=False)
    wmo_ext = nc.declare_dram_parameter("wmo", [D, R * D], BF16, isOutput=False)
    idx16_ext = nc.declare_dram_parameter("idx16", [128, TOT16], I16, isOutput=False)
    oall_ext = nc.declare_dram_parameter("oall", [TPC, 128, Cmax * 128], F8, isOutput=False)
    otall_ext = nc.declare_dram_parameter("otall", [TPC, 128, Cmax * 128], F8, isOutput=False)
    hexp_ext = nc.declare_dram_parameter("hexp", [NH, D], F32, isOutput=False)
    out_ext = nc.declare_dram_parameter("out", [NPC, D], F32, isOutput=True)

    kv_dram = nc.dram_tensor("kv_table", [N, 2 * D], BF16)

    NB = TT // 8               # phase-1 batches of 8 tiles (1024 nodes)
    Exp = mybir.ActivationFunctionType.Exp
    Copy = mybir.ActivationFunctionType.Copy

    with tile_mod.TileContext(nc) as tc:
        with (
            tc.tile_pool(name="const", bufs=1) as cp,
            tc.tile_pool(name="ph1", bufs=3) as p1,
            tc.tile_pool(name="qatp", bufs=4) as qp,
            tc.tile_pool(name="tile", bufs=3) as tp,
            tc.tile_pool(name="ps512", bufs=2, space="PSUM") as ps512,
            tc.tile_pool(name="ps_at", bufs=2, space="PSUM") as ps_at,
            tc.tile_pool(name="ps_sm", bufs=2, space="PSUM") as ps_sm,
        ):
            hexp_sb = cp.tile([NH, D], F32)
            nc.sync.dma_start(out=hexp_sb[:], in_=hexp_ext[:])
            ident = cp.tile([128, 128], F32)
            make_identity(nc, ident[:])
            wkv_sb = cp.tile([128, T * 256], BF16)
            nc.sync.dma_start(out=wkv_sb[:], in_=wkv_ext[:])
            wqa_sb = cp.tile([128, R * D], BF16)
            nc.sync.dma_start(out=wqa_sb[:], in_=wqa_ext[:])
            wmo_sb = cp.tile([128, R * D], BF16)
            nc.sync.dma_start(out=wmo_sb[:], in_=wmo_ext[:])
            ownht = cp.tile([128, NPC], BF16)
            nc.sync.dma_start(out=ownht[:], in_=ownht_ext[:])
            idx16_sb = cp.tile([128, TOT16], I16)
            nc.sync.dma_start(out=idx16_sb[:], in_=idx16_ext[:])

            # ---- phase 1: bf16 k|v table for all N nodes ----
            for b in range(NB):
                htc = p1.tile([128, 1024], BF16, tag="htc")
                nc.sync.dma_start(out=htc[:], in_=ht_ext[:, b * 1024:(b + 1) * 1024])
                kvs = p1.tile([128, 2048], BF16, tag="kvs")
                for i in range(4):
                    ty = (8 * b + 2 * i) // (NPT // 128)
                    kvp = ps512.tile([128, 512], F32, tag="p512")
                    for j in range(2):
                        nc.tensor.matmul(kvp[:, j * 256:(j + 1) * 256],
                                         lhsT=htc[:, (2 * i + j) * 128:(2 * i + j + 1) * 128],
                                         rhs=wkv_sb[:, ty * 256:(ty + 1) * 256],
                                         start=True, stop=True)
                    nc.scalar.activation(out=kvs[:, i * 512:(i + 1) * 512],
                                         in_=kvp[:], func=Copy)
                nc.sync.dma_start(
                    out=kv_dram[b * 1024:(b + 1) * 1024, :]
                        .rearrange("(t p) k -> p t k", p=128),
                    in_=kvs[:].rearrange("p (t k) -> p t k", t=8))

            # ---- phase 2: per node-tile edge processing ----
            qn = 0
            for tl in range(TPC):
                C = int(C_t[tl])
                CL = int(C_lo[tl])
                CH = int(C_hi[tl])
                rels = chunk_rel[tl]
                lo_off, hi_off = seg_off[tl]

                # per-tile rotated queries (stay in SBUF)
                qat = qp.tile([128, R * D], BF16, tag="qat")
                for i in range(2):
                    qah = ps512.tile([128, 512], F32, tag="p512")
                    nc.tensor.matmul(qah[:],
                                     lhsT=ownht[:, tl * 128:(tl + 1) * 128],
                                     rhs=wqa_sb[:, i * 512:(i + 1) * 512],
                                     start=True, stop=True)
                    nc.scalar.activation(out=qat[:, i * 512:(i + 1) * 512],
                                         in_=qah[:], func=Copy)

                # single_packet dma_gather caps at 64 descs/lane = 1024 idxs
                # = 8 chunks per op; split larger gathers into 8-chunk spans
                def gather_spans(dst, dst_col0, src_ap, idx_col0, nch, es):
                    nonlocal qn
                    for s0 in range(0, nch, 8):
                        sc = min(8, nch - s0)
                        nc.gpsimd.dma_gather(
                            dst[:, (dst_col0 + s0) * es:(dst_col0 + s0 + sc) * es]
                                .rearrange("p (c x) -> p c x", x=es),
                            src_ap,
                            idx16_sb[:, idx_col0 + s0 * 8:idx_col0 + (s0 + sc) * 8],
                            sc * 128, sc * 128, es, queue_num=qn % 4)
                        qn += 1

                kvg = tp.tile([128, Cmax * 256], BF16, tag="kvg")
                if CL:
                    gather_spans(kvg, 0, kv_dram[0:NHALF, :], lo_off, CL, 256)
                if CH:
                    gather_spans(kvg, CL, kv_dram[NHALF:N, :], hi_off, CH, 256)

                # one-hot O[e, (c,j)] and O^T[j, (c,e)] DMAd from host (fp8)
                Oall = tp.tile([128, Cmax * 128], F8, tag="Oall")
                nc.sync.dma_start(out=Oall[:, :C * 128], in_=oall_ext[tl, :, :C * 128])
                OT = tp.tile([128, Cmax * 128], F8, tag="OT")
                nc.sync.dma_start(out=OT[:, :C * 128], in_=otall_ext[tl, :, :C * 128])

                # qep[e, d] = qat[dst_e, rel_e, d] via one-hot matmuls, in
                # PSUM waves of 8 chunks; then attn = sum_d qep * k per head
                prod = tp.tile([128, Cmax * 128], BF16, tag="prod")
                for w0 in range(0, C, 4):
                    nw = min(4, C - w0)
                    qepw = ps512.tile([128, 512], F32, tag="p512")
                    for c in range(w0, w0 + nw):
                        rc = rels[c]
                        nc.tensor.matmul(qepw[:, (c - w0) * 128:(c - w0 + 1) * 128],
                                         lhsT=OT[:, c * 128:(c + 1) * 128],
                                         rhs=qat[:, rc * 128:(rc + 1) * 128],
                                         start=True, stop=True)
                    nc.vector.tensor_tensor(
                        out=prod[:, w0 * 128:(w0 + nw) * 128]
                            .rearrange("p (c d) -> p c d", c=nw),
                        in0=qepw[:, :nw * 128].rearrange("p (c d) -> p c d", c=nw),
                        in1=kvg[:, :C * 256].rearrange("p (c x) -> p c x", c=C)[:, w0:w0 + nw, 0:128],
                        op=mybir.AluOpType.mult,
                    )

                # pairwise tree-reduce of the 32 dk dims per head
                cur, n = prod, C * 128
                for lvl in range(4):
                    nxt = tp.tile([128, Cmax * (64 >> lvl)], BF16, tag=f"red{lvl}")
                    v = cur[:, :n].rearrange("p (g t) -> p g t", t=2)
                    nc.vector.tensor_tensor(
                        out=nxt[:, :n // 2].rearrange("p (g t) -> p g t", t=1),
                        in0=v[:, :, 0:1], in1=v[:, :, 1:2],
                        op=mybir.AluOpType.add,
                    )
                    cur, n = nxt, n // 2
                attn = tp.tile([128, Cmax * NH], F32, tag="attn")
                v = cur[:, :n].rearrange("p (g t) -> p g t", t=2)
                nc.vector.tensor_tensor(
                    out=attn[:, :C * NH].rearrange("p (g t) -> p g t", t=1),
                    in0=v[:, :, 0:1], in1=v[:, :, 1:2],
                    op=mybir.AluOpType.add,
                )
                wv = tp.tile([128, Cmax * NH], BF16, tag="wv")
                nc.scalar.activation(out=wv[:, :C * NH], in_=attn[:, :C * NH], func=Exp)

                # wm[e, d] = w[e, h(d)] * v_raw[src_e, d]
                wmt = tp.tile([128, Cmax * 128], BF16, tag="wmt")
                nc.vector.tensor_tensor(
                    out=wmt[:, :C * 128].rearrange("p (c h d) -> p c h d", c=C, h=NH),
                    in0=kvg[:, :C * 256].rearrange("p (c x) -> p c x", c=C)[:, :, 128:256]
                        .rearrange("p c (h d) -> p c h d", h=NH),
                    in1=wv[:, :C * NH].rearrange("p (c h u) -> p c h u", c=C, u=1)
                        .to_broadcast([128, C, NH, DK]),
                    op=mybir.AluOpType.mult,
                )

                # segment sums into PSUM: A_T[d, j] per relation block + s[j, h]
                # PSUM start=True marks the whole 2KB zero region pending --
                # accumulation groups sharing a bank must run back-to-back,
                # so iterate chunks grouped by relation (data layout unchanged)
                ATp = ps_at.tile([128, R * D], F32, tag="ATp")
                sp = ps_sm.tile([128, 128], F32, tag="sm")
                order = sorted(range(C), key=lambda c: rels[c])
                for k, c in enumerate(order):
                    rc = rels[c]
                    first = (k == 0) or rels[order[k - 1]] != rc
                    last = (k == C - 1) or rels[order[k + 1]] != rc
                    nc.tensor.matmul(ATp[:, rc * D:(rc + 1) * D],
                                     lhsT=wmt[:, c * 128:(c + 1) * 128],
                                     rhs=Oall[:, c * 128:(c + 1) * 128],
                                     start=first, stop=last, skip_group_check=True)
                for c in range(C):
                    nc.tensor.matmul(sp[:, :NH], lhsT=Oall[:, c * 128:(c + 1) * 128],
                                     rhs=wv[:, c * NH:(c + 1) * NH],
                                     start=(c == 0), stop=(c == C - 1),
                                     skip_group_check=True)

                rec = tp.tile([128, NH], F32, tag="rec")
                nc.vector.reciprocal(rec[:], sp[:, :NH])
                # rts[d, j] = rec[j, h(d)] via tiny transpose + K=4 matmul
                # against the constant head-expander hexp[h, d] = (h(d) == h)
                rtp = ps_sm.tile([128, 128], F32, tag="sm")
                nc.tensor.transpose(rtp[:NH, :], rec[:], ident[:])
                recT = tp.tile([NH, 128], F32, tag="recT")
                nc.scalar.activation(out=recT[:], in_=rtp[:NH, :], func=Copy)
                rts2 = ps_sm.tile([128, 128], F32, tag="sm")
                nc.tensor.matmul(rts2[:], lhsT=hexp_sb[:], rhs=recT[:],
                                 start=True, stop=True)
                rts = tp.tile([128, 128], F32, tag="rts")
                nc.scalar.activation(out=rts[:], in_=rts2[:], func=Copy)

                Anorm = tp.tile([128, R * D], BF16, tag="Anorm")
                nc.vector.tensor_tensor(
                    out=Anorm[:].rearrange("p (r j) -> p r j", r=R),
                    in0=ATp[:].rearrange("p (r j) -> p r j", r=R),
                    in1=rts[:].rearrange("p (u j) -> p u j", u=1).to_broadcast([128, R, 128]),
                    op=mybir.AluOpType.mult,
                )

                outp = ps_sm.tile([128, 128], F32, tag="sm")
                for r in range(R):
                    nc.tensor.matmul(outp[:], lhsT=Anorm[:, r * D:(r + 1) * D],
                                     rhs=wmo_sb[:, r * D:(r + 1) * D],
                                     start=(r == 0), stop=(r == R - 1))
                osb = tp.tile([128, 128], F32, tag="osb")
                nc.scalar.activation(out=osb[:], in_=outp[:], func=Copy)
                nc.sync.dma_start(out=out_ext[tl * 128:(tl + 1) * 128, :], in_=osb[:])
    nc.compile()
    return nc


def kernel(h, k_linears, q_linears, v_linears, a_linears,
           relation_att, relation_msg, relation_pri, skip,
           row_idx, col_idx, eids, **_unused):
    in_maps, meta = _host_prep(
        h, k_linears, q_linears, v_linears, a_linears,
        relation_att, relation_msg, relation_pri, skip, row_idx, col_idx)
    nc = _build_program(meta)
    kw = {}
    if os.environ.get("KBENCH_TRACE"):
        kw = dict(trace=True, tmpdir=os.environ.get("KBENCH_TMPDIR") or None)
    res = run_bass_kernel_spmd(nc, in_maps, list(range(NCORES)), **kw)
    global LAST_RESULTS
    LAST_RESULTS = res
    out = np.concatenate([res.results[c]["out"] for c in range(NCORES)], axis=0)
    return out.astype(np.float32)


LAST_RESULTS = None


# revision 43
# speedup vs baseline: 1.2053x; 1.0072x over previous
"""HGT (heterogeneous graph transformer) layer on 8 trn2 NeuronCores.

Strategy (dst-node 1D sharding, uniform SPMD program):
  - Host folds all small weights:
      WKV[t]    = [W_k[t] | W_v[t]]                      (node-type projections)
      WQA[t,r]  = W_q[t] @ blockdiag(W_att[r] * pri[r,h]/sqrt(dk))
      WMO[r,t]  = blockdiag(W_msg[r]) @ (sigmoid(skip[t])*W_a[t])
    so the per-edge computation needs only RAW k/v of the src node:
      attn[e,h] = qat[rel][dst] . k_raw[src]     (per head, pri pre-folded)
      agg[j]    = sum_r (sum_{e in rel r, dst=j} w_e * v_raw[src]) @ WMO[r]
      out[j]    = agg[j] / s[j]                  (softmax denominator)
  - Each core owns a contiguous range of 6400 dst nodes (single node type).
    Per core the edges are grouped into (node-tile of 128 dst, relation,
    src-half, chunk of 128 edges); chunk structure is the max over cores so
    the SPMD program is identical on all cores, with per-core data padded.
    The src-half split (src < 25600 vs >=) keeps gather indices within
    int16 range for the batched SWDGE dma_gather instruction.
  - All matmul inputs are bf16 (4x PE rate vs fp32); PSUM accumulates fp32.
    The one-hot (edge,dst) matrices O and O^T are precomputed on host and
    shipped as fp8 (0/1 exact).
  - Host pre-transposes h to hT [128, N] bf16, so no PE transposes for the
    projections.  Phase 1 builds the bf16 [N,256] k|v table with batched
    1024-node DMAs; per tile, the rotated queries qat stay in SBUF.
  - Per node-tile, batched dma_gather ops (<=1024 indices each, the
    single-packet 64-descs/lane cap) fetch all edges' k|v rows, spread
    round-robin over the 4 SWDGE queues; a monkeypatched DMASW-lane
    assignment pins each queue to its own semaphore lanes.
  - Per-edge q is gathered on the PE: qep = O^T @ qat into PSUM waves;
    attn = pairwise tree-reduce of qep*k (grouped TENSOR_REDUCE is slow).
  - Segment sums over dst are one-hot matmuls into PSUM accumulated per
    relation; PSUM start=True marks the whole 2KB zero region pending, so
    accumulation groups sharing a bank are iterated back-to-back per rel.
  - The softmax denominator reciprocal is broadcast dk-wise via a tiny
    transpose + K=4 matmul against a constant head-expander matrix.
  - Padded edge slots have all-zero one-hot rows -> zero contribution to
    both numerator and denominator; their gathered values are real table
    rows so no NaN risk.
  - Softmax skips the segment-max subtraction: scores are O(1) here, and
    exp(s)/sum(exp(s)) is invariant to the shift.
"""

import os
import sys

sys.path.insert(0, "/opt/trn_rl_repo")

import ml_dtypes
import numpy as np

import concourse.bass as bass
import concourse.bacc as bacc_mod
import concourse.mybir as mybir
import concourse.tile as tile_mod
import concourse.tile_sem_assignment as _tsa
from concourse.bass_utils import run_bass_kernel_spmd
from concourse.masks import make_identity

# The tile framework rotates SWDGE DMAs over the 8 DMASW semaphore lanes in
# scheduled order, but each physical semaphore is locked to one SWDGE queue.
# To run gathers on all 4 queues concurrently, pin each queue to its own pair
# of lanes so a semaphore only ever sees one queue.
_ORIG_ASSIGN_TICK = _tsa.TileClockTick._assign_tick


def _qlane_assign_tick(self, inst):
    qnum = getattr(inst, "queue_num", None)
    if (qnum is not None and inst.engine == mybir.EngineType.Pool
            and isinstance(inst, _tsa.DMAInst)
            and not isinstance(inst, _tsa.bass_isa.UserSyncedRemoteDMADescs)
            and self.swdge_sem_count >= 8):
        cnt = getattr(self, "_qlane_cnt", None)
        if cnt is None:
            cnt = self._qlane_cnt = [0, 0, 0, 0]
        lanes = self.swdge_sem_count // 4
        save = self.next_sw_dma_idx
        self.next_sw_dma_idx = qnum * lanes + (cnt[qnum] % lanes)
        cnt[qnum] += 1
        try:
            return _ORIG_ASSIGN_TICK(self, inst)
        finally:
            self.next_sw_dma_idx = save
    return _ORIG_ASSIGN_TICK(self, inst)


_tsa.TileClockTick._assign_tick = _qlane_assign_tick

F32 = mybir.dt.float32
BF16 = mybir.dt.bfloat16
F8 = mybir.dt.float8e4
I16 = mybir.dt.int16
NPBF16 = ml_dtypes.bfloat16
NPF8 = ml_dtypes.float8_e4m3

N, E, T, R, NH, DK, D = 51200, 640000, 4, 8, 4, 32, 128
NCORES = 8
NPC = N // NCORES          # 6400 nodes per core
TPC = NPC // 128           # 50 node-tiles per core
TT = N // 128              # 400 table tiles
NPT = N // T               # nodes per type
EPR = E // R               # edges per relation
NHALF = N // 2             # src-half split for int16 gather indices
SQRT_DK = float(np.sqrt(DK))


def _blockdiag(W):
    """[R,H,dk,dk] -> [R,D,D] block-diagonal per head."""
    out = np.zeros((R, D, D), np.float32)
    for r in range(R):
        for hh in range(NH):
            out[r, hh * DK:(hh + 1) * DK, hh * DK:(hh + 1) * DK] = W[r, hh]
    return out


def _wrap16(L):
    """Linear int16 index list -> [128, ceil(n/16)] SBUF layout (wrapped in 16
    partitions, replicated to all 8 16-partition groups)."""
    n = L.shape[0]
    F = -(-n // 16)
    pad = np.zeros(F * 16, np.int16)
    pad[:n] = L
    seg = pad.reshape(F, 16).T               # [16, F]
    return np.tile(seg, (8, 1))              # [128, F]


def _host_prep(h, k_linears, q_linears, v_linears, a_linears,
               relation_att, relation_msg, relation_pri, skip,
               row_idx, col_idx):
    pri = np.asarray(relation_pri, np.float32) / SQRT_DK               # [R,H]
    att = np.asarray(relation_att, np.float32) * pri[:, :, None, None]
    Watt = _blockdiag(att)
    Wmsg = _blockdiag(np.asarray(relation_msg, np.float32))
    skip = np.asarray(skip, np.float32)
    Wout = (1.0 / (1.0 + np.exp(-skip))).astype(np.float32) * np.asarray(a_linears, np.float32)
    WQA = np.einsum("tab,rbc->trac", np.asarray(q_linears, np.float32), Watt)
    WMO = np.einsum("rab,tbc->rtac", Wmsg, Wout)
    WKV = np.concatenate([np.asarray(k_linears, np.float32),
                          np.asarray(v_linears, np.float32)], axis=2)  # [T,D,256]

    row = np.asarray(row_idx, np.int64)
    col = np.asarray(col_idx, np.int64)
    erel = np.arange(E, dtype=np.int64) // EPR
    half = (row >= NHALF).astype(np.int64)

    core = col // NPC
    tl = (col % NPC) // 128
    # per-(core,tile,rel,half) edge counts
    key = ((core * TPC + tl) * R + erel) * 2 + half
    counts = np.bincount(key, minlength=NCORES * TPC * R * 2).reshape(NCORES, TPC, R, 2)
    maxcnt = counts.max(axis=0)                                       # [TPC,R,2]
    n_chunks = -(-maxcnt // 128)                                      # ceil
    # ensure at least one chunk per tile (degenerate safety)
    C_lo_t = n_chunks[:, :, 0].sum(axis=1)
    C_hi_t = n_chunks[:, :, 1].sum(axis=1)
    # chunk index base per (tile, rel, half): lo chunks first, then hi
    chunk_base = np.zeros((TPC, R, 2), np.int64)
    for t in range(TPC):
        off = 0
        for r in range(R):
            chunk_base[t, r, 0] = off
            off += n_chunks[t, r, 0]
        for r in range(R):
            chunk_base[t, r, 1] = off
            off += n_chunks[t, r, 1]
    C_t = C_lo_t + C_hi_t
    Cmax = int(C_t.max())

    # per-core padded metadata arrays
    idx_all = np.zeros((NCORES, TPC, 128, Cmax), np.int16)   # kv idx (half-local)
    idx2_all = np.zeros((NCORES, TPC, 128, Cmax), np.int16)  # qat idx (tile-local)
    rds_all = np.full((NCORES, TPC, 128, Cmax), 999.0, NPBF16)

    order = np.argsort(key, kind="stable")
    ranks = np.empty(E, np.int64)
    group_start = np.zeros(NCORES * TPC * R * 2, np.int64)
    cnt_flat = counts.reshape(-1)
    np.cumsum(cnt_flat[:-1], out=group_start[1:])
    ranks[order] = np.arange(E) - group_start[key[order]]

    chunk_of = chunk_base[tl, erel, half] + ranks // 128              # [E]
    part_of = ranks % 128
    rd = col % 128
    idx_all[core, tl, part_of, chunk_of] = (row - half * NHALF).astype(np.int16)
    idx2_all[core, tl, part_of, chunk_of] = (erel * 128 + rd).astype(np.int16)
    rds_all[core, tl, part_of, chunk_of] = rd.astype(NPBF16)

    # chunk -> relation map per tile (lo section then hi section)
    chunk_rel = []
    for t in range(TPC):
        rels = []
        for hh in range(2):
            for r in range(R):
                rels += [r] * int(n_chunks[t, r, hh])
        chunk_rel.append(rels)

    # int16 gather index lists, wrap16 layout, concatenated per core:
    # per tile: [kv-lo (C_lo*8) | kv-hi (C_hi*8)] columns
    idx16 = np.zeros((NCORES, 128, 8 * int(C_t.sum())), np.int16)
    seg_off = []          # per tile: (lo_off, hi_off) in i16 columns
    off = 0
    for t in range(TPC):
        clo, chi, ct = int(C_lo_t[t]), int(C_hi_t[t]), int(C_t[t])
        seg_off.append((off, off + clo * 8))
        for c in range(NCORES):
            g = idx_all[c, t, :, :ct]          # [128, C]
            lo = _wrap16(np.ascontiguousarray(g[:, :clo].T).reshape(-1))
            hi = _wrap16(np.ascontiguousarray(g[:, clo:ct].T).reshape(-1))
            idx16[c, :, off:off + clo * 8] = lo
            idx16[c, :, off + clo * 8:off + ct * 8] = hi
        off += ct * 8
    idx16 = np.ascontiguousarray(idx16[:, :, :off])
    TOT16 = off

    # one-hot O[e, (c, j)] and O^T[j, (c, e)] shipped directly as fp8
    # (0/1 exact); padded slots (rds=999) give all-zero rows/cols
    jj = np.arange(128)
    o_all = np.zeros((NCORES, TPC, 128, Cmax * 128), NPF8)
    ot_all = np.zeros((NCORES, TPC, 128, Cmax * 128), NPF8)
    rds_f = rds_all.astype(np.float32)
    for c in range(NCORES):
        for t in range(TPC):
            oh = (rds_f[c, t][:, :, None] == jj[None, None, :])   # [e, C, j]
            o_all[c, t] = oh.reshape(128, -1).astype(NPF8)
            ot_all[c, t] = oh.transpose(2, 1, 0).reshape(128, -1).astype(NPF8)

    hT = np.ascontiguousarray(np.asarray(h, np.float32).T.astype(NPBF16))  # [128, N]
    hexp = np.zeros((NH, D), np.float32)      # head expander: hexp[h, d] = (d//DK == h)
    for hh in range(NH):
        hexp[hh, hh * DK:(hh + 1) * DK] = 1.0


    in_maps = []
    for c in range(NCORES):
        t_c = (c * NPC) // NPT
        in_maps.append({
            "ht": hT,
            "ownht": np.ascontiguousarray(hT[:, c * NPC:(c + 1) * NPC]),
            "wkv": np.ascontiguousarray(
                WKV.transpose(1, 0, 2).reshape(D, T * 256).astype(NPBF16)),
            "wqa": np.ascontiguousarray(
                WQA[t_c].transpose(1, 0, 2).reshape(D, R * D).astype(NPBF16)),
            "wmo": np.ascontiguousarray(
                WMO[:, t_c].transpose(1, 0, 2).reshape(D, R * D).astype(NPBF16)),
            "idx16": idx16[c],
            "oall": o_all[c],
            "otall": ot_all[c],
            "hexp": hexp,
        })
    meta = dict(chunk_rel=chunk_rel, C_lo=C_lo_t, C_hi=C_hi_t, C_t=C_t,
                Cmax=Cmax, seg_off=seg_off, TOT16=TOT16)
    return in_maps, meta


def _build_program(meta):
    chunk_rel, C_lo, C_hi, C_t = (meta["chunk_rel"], meta["C_lo"],
                                  meta["C_hi"], meta["C_t"])
    Cmax, seg_off, TOT16 = meta["Cmax"], meta["seg_off"], meta["TOT16"]

    nc = bacc_mod.Bacc(num_swdge_queues=4)
    ht_ext = nc.declare_dram_parameter("ht", [D, N], BF16, isOutput=False)
    ownht_ext = nc.declare_dram_parameter("ownht", [D, NPC], BF16, isOutput=False)
    wkv_ext = nc.declare_dram_parameter("wkv", [D, T * 256], BF16, isOutput=False)
    wqa_ext = nc.declare_dram_parameter("wqa", [D, R * D], BF16, isOutput=False)
    wmo_ext = nc.declare_dram_parameter("wmo", [D, R * D], BF16, isOutput=False)
    idx16_ext = nc.declare_dram_parameter("idx16", [128, TOT16], I16, isOutput=False)
    oall_ext = nc.declare_dram_parameter("oall", [TPC, 128, Cmax * 128], F8, isOutput=False)
    otall_ext = nc.declare_dram_parameter("otall", [TPC, 128, Cmax * 128], F8, isOutput=False)
    hexp_ext = nc.declare_dram_parameter("hexp", [NH, D], F32, isOutput=False)
    out_ext = nc.declare_dram_parameter("out", [NPC, D], F32, isOutput=True)

    kv_dram = nc.dram_tensor("kv_table", [N, 2 * D], BF16)

    NB = TT // 8               # phase-1 batches of 8 tiles (1024 nodes)
    Exp = mybir.ActivationFunctionType.Exp
    Copy = mybir.ActivationFunctionType.Copy

    with tile_mod.TileContext(nc) as tc:
        with (
            tc.tile_pool(name="const", bufs=1) as cp,
            tc.tile_pool(name="ph1", bufs=3) as p1,
            tc.tile_pool(name="qatp", bufs=4) as qp,
            tc.tile_pool(name="tile", bufs=4) as tp,
            tc.tile_pool(name="ps512", bufs=2, space="PSUM") as ps512,
            tc.tile_pool(name="ps_at", bufs=2, space="PSUM") as ps_at,
            tc.tile_pool(name="ps_sm", bufs=2, space="PSUM") as ps_sm,
        ):
            hexp_sb = cp.tile([NH, D], F32)
            nc.sync.dma_start(out=hexp_sb[:], in_=hexp_ext[:])
            ident = cp.tile([128, 128], F32)
            make_identity(nc, ident[:])
            wkv_sb = cp.tile([128, T * 256], BF16)
            nc.sync.dma_start(out=wkv_sb[:], in_=wkv_ext[:])
            wqa_sb = cp.tile([128, R * D], BF16)
            nc.sync.dma_start(out=wqa_sb[:], in_=wqa_ext[:])
            wmo_sb = cp.tile([128, R * D], BF16)
            nc.sync.dma_start(out=wmo_sb[:], in_=wmo_ext[:])
            ownht = cp.tile([128, NPC], BF16)
            nc.sync.dma_start(out=ownht[:], in_=ownht_ext[:])
            idx16_sb = cp.tile([128, TOT16], I16)
            nc.sync.dma_start(out=idx16_sb[:], in_=idx16_ext[:])

            # ---- phase 1: bf16 k|v table for all N nodes ----
            for b in range(NB):
                htc = p1.tile([128, 1024], BF16, tag="htc")
                nc.sync.dma_start(out=htc[:], in_=ht_ext[:, b * 1024:(b + 1) * 1024])
                kvs = p1.tile([128, 2048], BF16, tag="kvs")
                for i in range(4):
                    ty = (8 * b + 2 * i) // (NPT // 128)
                    kvp = ps512.tile([128, 512], F32, tag="p512")
                    for j in range(2):
                        nc.tensor.matmul(kvp[:, j * 256:(j + 1) * 256],
                                         lhsT=htc[:, (2 * i + j) * 128:(2 * i + j + 1) * 128],
                                         rhs=wkv_sb[:, ty * 256:(ty + 1) * 256],
                                         start=True, stop=True)
                    if i % 2 == 0:
                        nc.scalar.activation(out=kvs[:, i * 512:(i + 1) * 512],
                                             in_=kvp[:], func=Copy)
                    else:
                        nc.vector.tensor_copy(kvs[:, i * 512:(i + 1) * 512], kvp[:])
                nc.sync.dma_start(
                    out=kv_dram[b * 1024:(b + 1) * 1024, :]
                        .rearrange("(t p) k -> p t k", p=128),
                    in_=kvs[:].rearrange("p (t k) -> p t k", t=8))

            # ---- phase 2: per node-tile edge processing ----
            qn = 0
            for tl in range(TPC):
                C = int(C_t[tl])
                CL = int(C_lo[tl])
                CH = int(C_hi[tl])
                rels = chunk_rel[tl]
                lo_off, hi_off = seg_off[tl]

                # per-tile rotated queries (stay in SBUF)
                qat = qp.tile([128, R * D], BF16, tag="qat")
                for i in range(2):
                    qah = ps512.tile([128, 512], F32, tag="p512")
                    nc.tensor.matmul(qah[:],
                                     lhsT=ownht[:, tl * 128:(tl + 1) * 128],
                                     rhs=wqa_sb[:, i * 512:(i + 1) * 512],
                                     start=True, stop=True)
                    nc.scalar.activation(out=qat[:, i * 512:(i + 1) * 512],
                                         in_=qah[:], func=Copy)

                # single_packet dma_gather caps at 64 descs/lane = 1024 idxs
                # = 8 chunks per op; split larger gathers into 8-chunk spans
                def gather_spans(dst, dst_col0, src_ap, idx_col0, nch, es):
                    nonlocal qn
                    for s0 in range(0, nch, 8):
                        sc = min(8, nch - s0)
                        nc.gpsimd.dma_gather(
                            dst[:, (dst_col0 + s0) * es:(dst_col0 + s0 + sc) * es]
                                .rearrange("p (c x) -> p c x", x=es),
                            src_ap,
                            idx16_sb[:, idx_col0 + s0 * 8:idx_col0 + (s0 + sc) * 8],
                            sc * 128, sc * 128, es, queue_num=qn % 4)
                        qn += 1

                kvg = tp.tile([128, Cmax * 256], BF16, tag="kvg")
                if CL:
                    gather_spans(kvg, 0, kv_dram[0:NHALF, :], lo_off, CL, 256)
                if CH:
                    gather_spans(kvg, CL, kv_dram[NHALF:N, :], hi_off, CH, 256)

                # one-hot O[e, (c,j)] and O^T[j, (c,e)] DMAd from host (fp8)
                Oall = tp.tile([128, Cmax * 128], F8, tag="Oall")
                nc.sync.dma_start(out=Oall[:, :C * 128], in_=oall_ext[tl, :, :C * 128])
                OT = tp.tile([128, Cmax * 128], F8, tag="OT")
                nc.sync.dma_start(out=OT[:, :C * 128], in_=otall_ext[tl, :, :C * 128])

                # qep[e, d] = qat[dst_e, rel_e, d] via one-hot matmuls, in
                # PSUM waves of 8 chunks; then attn = sum_d qep * k per head
                prod = tp.tile([128, Cmax * 128], BF16, tag="prod")
                for w0 in range(0, C, 4):
                    nw = min(4, C - w0)
                    qepw = ps512.tile([128, 512], F32, tag="p512")
                    for c in range(w0, w0 + nw):
                        rc = rels[c]
                        nc.tensor.matmul(qepw[:, (c - w0) * 128:(c - w0 + 1) * 128],
                                         lhsT=OT[:, c * 128:(c + 1) * 128],
                                         rhs=qat[:, rc * 128:(rc + 1) * 128],
                                         start=True, stop=True)
                    nc.vector.tensor_tensor(
                        out=prod[:, w0 * 128:(w0 + nw) * 128]
                            .rearrange("p (c d) -> p c d", c=nw),
                        in0=qepw[:, :nw * 128].rearrange("p (c d) -> p c d", c=nw),
                        in1=kvg[:, :C * 256].rearrange("p (c x) -> p c x", c=C)[:, w0:w0 + nw, 0:128],
                        op=mybir.AluOpType.mult,
                    )

                # pairwise tree-reduce of the 32 dk dims per head
                cur, n = prod, C * 128
                for lvl in range(4):
                    nxt = tp.tile([128, Cmax * (64 >> lvl)], BF16, tag=f"red{lvl}")
                    v = cur[:, :n].rearrange("p (g t) -> p g t", t=2)
                    nc.vector.tensor_tensor(
                        out=nxt[:, :n // 2].rearrange("p (g t) -> p g t", t=1),
                        in0=v[:, :, 0:1], in1=v[:, :, 1:2],
                        op=mybir.AluOpType.add,
                    )
                    cur, n = nxt, n // 2
                attn = tp.tile([128, Cmax * NH], F32, tag="attn")
                v = cur[:, :n].rearrange("p (g t) -> p g t", t=2)
                nc.vector.tensor_tensor(
                    out=attn[:, :C * NH].rearrange("p (g t) -> p g t", t=1),
                    in0=v[:, :, 0:1], in1=v[:, :, 1:2],
                    op=mybir.AluOpType.add,
                )
                wv = tp.tile([128, Cmax * NH], BF16, tag="wv")
                nc.scalar.activation(out=wv[:, :C * NH], in_=attn[:, :C * NH], func=Exp)

                # wm[e, d] = w[e, h(d)] * v_raw[src_e, d]
                wmt = tp.tile([128, Cmax * 128], BF16, tag="wmt")
                nc.vector.tensor_tensor(
                    out=wmt[:, :C * 128].rearrange("p (c h d) -> p c h d", c=C, h=NH),
                    in0=kvg[:, :C * 256].rearrange("p (c x) -> p c x", c=C)[:, :, 128:256]
                        .rearrange("p c (h d) -> p c h d", h=NH),
                    in1=wv[:, :C * NH].rearrange("p (c h u) -> p c h u", c=C, u=1)
                        .to_broadcast([128, C, NH, DK]),
                    op=mybir.AluOpType.mult,
                )

                # segment sums into PSUM: A_T[d, j] per relation block + s[j, h]
                # PSUM start=True marks the whole 2KB zero region pending --
                # accumulation groups sharing a bank must run back-to-back,
                # so iterate chunks grouped by relation (data layout unchanged)
                ATp = ps_at.tile([128, R * D], F32, tag="ATp")
                sp = ps_sm.tile([128, 128], F32, tag="sm")
                order = sorted(range(C), key=lambda c: rels[c])
                for k, c in enumerate(order):
                    rc = rels[c]
                    first = (k == 0) or rels[order[k - 1]] != rc
                    last = (k == C - 1) or rels[order[k + 1]] != rc
                    nc.tensor.matmul(ATp[:, rc * D:(rc + 1) * D],
                                     lhsT=wmt[:, c * 128:(c + 1) * 128],
                                     rhs=Oall[:, c * 128:(c + 1) * 128],
                                     start=first, stop=last, skip_group_check=True)
                for c in range(C):
                    nc.tensor.matmul(sp[:, :NH], lhsT=Oall[:, c * 128:(c + 1) * 128],
                                     rhs=wv[:, c * NH:(c + 1) * NH],
                                     start=(c == 0), stop=(c == C - 1),
                                     skip_group_check=True)

                rec = tp.tile([128, NH], F32, tag="rec")
                nc.vector.reciprocal(rec[:], sp[:, :NH])
                # rts[d, j] = rec[j, h(d)] via tiny transpose + K=4 matmul
                # against the constant head-expander hexp[h, d] = (h(d) == h)
                rtp = ps_sm.tile([128, 128], F32, tag="sm")
                nc.tensor.transpose(rtp[:NH, :], rec[:], ident[:])
                recT = tp.tile([NH, 128], F32, tag="recT")
                nc.scalar.activation(out=recT[:], in_=rtp[:NH, :], func=Copy)
                rts2 = ps_sm.tile([128, 128], F32, tag="sm")
                nc.tensor.matmul(rts2[:], lhsT=hexp_sb[:], rhs=recT[:],
                                 start=True, stop=True)
                rts = tp.tile([128, 128], F32, tag="rts")
                nc.scalar.activation(out=rts[:], in_=rts2[:], func=Copy)

                Anorm = tp.tile([128, R * D], BF16, tag="Anorm")
                nc.vector.tensor_tensor(
                    out=Anorm[:].rearrange("p (r j) -> p r j", r=R),
                    in0=ATp[:].rearrange("p (r j) -> p r j", r=R),
                    in1=rts[:].rearrange("p (u j) -> p u j", u=1).to_broadcast([128, R, 128]),
                    op=mybir.AluOpType.mult,
                )

                outp = ps_sm.tile([128, 128], F32, tag="sm")
                for r in range(R):
                    nc.tensor.matmul(outp[:], lhsT=Anorm[:, r * D:(r + 1) * D],
                                     rhs=wmo_sb[:, r * D:(r + 1) * D],
                                     start=(r == 0), stop=(r == R - 1))
                osb = tp.tile([128, 128], F32, tag="osb")
                nc.scalar.activation(out=osb[:], in_=outp[:], func=Copy)
                nc.sync.dma_start(out=out_ext[tl * 128:(tl + 1) * 128, :], in_=osb[:])
    nc.compile()
    return nc


def kernel(h, k_linears, q_linears, v_linears, a_linears,
           relation_att, relation_msg, relation_pri, skip,
           row_idx, col_idx, eids, **_unused):
    in_maps, meta = _host_prep(
        h, k_linears, q_linears, v_linears, a_linears,
        relation_att, relation_msg, relation_pri, skip, row_idx, col_idx)
    nc = _build_program(meta)
    kw = {}
    if os.environ.get("KBENCH_TRACE"):
        kw = dict(trace=True, tmpdir=os.environ.get("KBENCH_TMPDIR") or None)
    res = run_bass_kernel_spmd(nc, in_maps, list(range(NCORES)), **kw)
    global LAST_RESULTS
    LAST_RESULTS = res
    out = np.concatenate([res.results[c]["out"] for c in range(NCORES)], axis=0)
    return out.astype(np.float32)


LAST_RESULTS = None


# revision 44
# speedup vs baseline: 1.2295x; 1.0201x over previous
"""HGT (heterogeneous graph transformer) layer on 8 trn2 NeuronCores.

Strategy (dst-node 1D sharding, uniform SPMD program):
  - Host folds all small weights:
      WKV[t]    = [W_k[t] | W_v[t]]                      (node-type projections)
      WQA[t,r]  = W_q[t] @ blockdiag(W_att[r] * pri[r,h]/sqrt(dk))
      WMO[r,t]  = blockdiag(W_msg[r]) @ (sigmoid(skip[t])*W_a[t])
    so the per-edge computation needs only RAW k/v of the src node:
      attn[e,h] = qat[rel][dst] . k_raw[src]     (per head, pri pre-folded)
      agg[j]    = sum_r (sum_{e in rel r, dst=j} w_e * v_raw[src]) @ WMO[r]
      out[j]    = agg[j] / s[j]                  (softmax denominator)
  - Each core owns a contiguous range of 6400 dst nodes (single node type).
    Per core the edges are grouped into (node-tile of 128 dst, relation,
    src-half, chunk of 128 edges); chunk structure is the max over cores so
    the SPMD program is identical on all cores, with per-core data padded.
    The src-half split (src < 25600 vs >=) keeps gather indices within
    int16 range for the batched SWDGE dma_gather instruction.
  - All matmul inputs are bf16 (4x PE rate vs fp32); PSUM accumulates fp32.
    The one-hot (edge,dst) matrices O and O^T are precomputed on host and
    shipped as fp8 (0/1 exact).
  - Host pre-transposes h to hT [128, N] bf16, so no PE transposes for the
    projections.  Phase 1 builds the bf16 [N,256] k|v table with batched
    1024-node DMAs; per tile, the rotated queries qat stay in SBUF.
  - Per node-tile, batched dma_gather ops (<=1024 indices each, the
    single-packet 64-descs/lane cap) fetch all edges' k|v rows, spread
    round-robin over the 4 SWDGE queues; a monkeypatched DMASW-lane
    assignment pins each queue to its own semaphore lanes.
  - Per-edge q is gathered on the PE: qep = O^T @ qat into PSUM waves;
    attn = pairwise tree-reduce of qep*k (grouped TENSOR_REDUCE is slow).
  - Segment sums over dst are one-hot matmuls into PSUM accumulated per
    relation; PSUM start=True marks the whole 2KB zero region pending, so
    accumulation groups sharing a bank are iterated back-to-back per rel.
  - The softmax denominator reciprocal is broadcast dk-wise via a tiny
    transpose + K=4 matmul against a constant head-expander matrix.
  - Padded edge slots have all-zero one-hot rows -> zero contribution to
    both numerator and denominator; their gathered values are real table
    rows so no NaN risk.
  - Softmax skips the segment-max subtraction: scores are O(1) here, and
    exp(s)/sum(exp(s)) is invariant to the shift.
"""

import os
import sys

sys.path.insert(0, "/opt/trn_rl_repo")

import ml_dtypes
import numpy as np

import concourse.bass as bass
import concourse.bacc as bacc_mod
import concourse.mybir as mybir
import concourse.tile as tile_mod
import concourse.tile_sem_assignment as _tsa
from concourse.bass_utils import run_bass_kernel_spmd
from concourse.masks import make_identity

# The tile framework rotates SWDGE DMAs over the 8 DMASW semaphore lanes in
# scheduled order, but each physical semaphore is locked to one SWDGE queue.
# To run gathers on all 4 queues concurrently, pin each queue to its own pair
# of lanes so a semaphore only ever sees one queue.
_ORIG_ASSIGN_TICK = _tsa.TileClockTick._assign_tick


def _qlane_assign_tick(self, inst):
    qnum = getattr(inst, "queue_num", None)
    if (qnum is not None and inst.engine == mybir.EngineType.Pool
            and isinstance(inst, _tsa.DMAInst)
            and not isinstance(inst, _tsa.bass_isa.UserSyncedRemoteDMADescs)
            and self.swdge_sem_count >= 8):
        cnt = getattr(self, "_qlane_cnt", None)
        if cnt is None:
            cnt = self._qlane_cnt = [0, 0, 0, 0]
        lanes = self.swdge_sem_count // 4
        save = self.next_sw_dma_idx
        self.next_sw_dma_idx = qnum * lanes + (cnt[qnum] % lanes)
        cnt[qnum] += 1
        try:
            return _ORIG_ASSIGN_TICK(self, inst)
        finally:
            self.next_sw_dma_idx = save
    return _ORIG_ASSIGN_TICK(self, inst)


_tsa.TileClockTick._assign_tick = _qlane_assign_tick

F32 = mybir.dt.float32
BF16 = mybir.dt.bfloat16
F8 = mybir.dt.float8e4
I16 = mybir.dt.int16
NPBF16 = ml_dtypes.bfloat16
NPF8 = ml_dtypes.float8_e4m3

N, E, T, R, NH, DK, D = 51200, 640000, 4, 8, 4, 32, 128
NCORES = 8
NPC = N // NCORES          # 6400 nodes per core
TPC = NPC // 128           # 50 node-tiles per core
TT = N // 128              # 400 table tiles
NPT = N // T               # nodes per type
EPR = E // R               # edges per relation
NHALF = N // 2             # src-half split for int16 gather indices
SQRT_DK = float(np.sqrt(DK))


def _blockdiag(W):
    """[R,H,dk,dk] -> [R,D,D] block-diagonal per head."""
    out = np.zeros((R, D, D), np.float32)
    for r in range(R):
        for hh in range(NH):
            out[r, hh * DK:(hh + 1) * DK, hh * DK:(hh + 1) * DK] = W[r, hh]
    return out


def _wrap16(L):
    """Linear int16 index list -> [128, ceil(n/16)] SBUF layout (wrapped in 16
    partitions, replicated to all 8 16-partition groups)."""
    n = L.shape[0]
    F = -(-n // 16)
    pad = np.zeros(F * 16, np.int16)
    pad[:n] = L
    seg = pad.reshape(F, 16).T               # [16, F]
    return np.tile(seg, (8, 1))              # [128, F]


def _host_prep(h, k_linears, q_linears, v_linears, a_linears,
               relation_att, relation_msg, relation_pri, skip,
               row_idx, col_idx):
    pri = np.asarray(relation_pri, np.float32) / SQRT_DK               # [R,H]
    att = np.asarray(relation_att, np.float32) * pri[:, :, None, None]
    Watt = _blockdiag(att)
    Wmsg = _blockdiag(np.asarray(relation_msg, np.float32))
    skip = np.asarray(skip, np.float32)
    Wout = (1.0 / (1.0 + np.exp(-skip))).astype(np.float32) * np.asarray(a_linears, np.float32)
    WQA = np.einsum("tab,rbc->trac", np.asarray(q_linears, np.float32), Watt)
    WMO = np.einsum("rab,tbc->rtac", Wmsg, Wout)
    WKV = np.concatenate([np.asarray(k_linears, np.float32),
                          np.asarray(v_linears, np.float32)], axis=2)  # [T,D,256]

    row = np.asarray(row_idx, np.int64)
    col = np.asarray(col_idx, np.int64)
    erel = np.arange(E, dtype=np.int64) // EPR
    half = (row >= NHALF).astype(np.int64)

    core = col // NPC
    tl = (col % NPC) // 128
    # per-(core,tile,rel,half) edge counts
    key = ((core * TPC + tl) * R + erel) * 2 + half
    counts = np.bincount(key, minlength=NCORES * TPC * R * 2).reshape(NCORES, TPC, R, 2)
    maxcnt = counts.max(axis=0)                                       # [TPC,R,2]
    n_chunks = -(-maxcnt // 128)                                      # ceil
    # ensure at least one chunk per tile (degenerate safety)
    C_lo_t = n_chunks[:, :, 0].sum(axis=1)
    C_hi_t = n_chunks[:, :, 1].sum(axis=1)
    # chunk index base per (tile, rel, half): lo chunks first, then hi
    chunk_base = np.zeros((TPC, R, 2), np.int64)
    for t in range(TPC):
        off = 0
        for r in range(R):
            chunk_base[t, r, 0] = off
            off += n_chunks[t, r, 0]
        for r in range(R):
            chunk_base[t, r, 1] = off
            off += n_chunks[t, r, 1]
    C_t = C_lo_t + C_hi_t
    Cmax = int(C_t.max())

    # per-core padded metadata arrays
    idx_all = np.zeros((NCORES, TPC, 128, Cmax), np.int16)   # kv idx (half-local)
    idx2_all = np.zeros((NCORES, TPC, 128, Cmax), np.int16)  # qat idx (tile-local)
    rds_all = np.full((NCORES, TPC, 128, Cmax), 999.0, NPBF16)

    order = np.argsort(key, kind="stable")
    ranks = np.empty(E, np.int64)
    group_start = np.zeros(NCORES * TPC * R * 2, np.int64)
    cnt_flat = counts.reshape(-1)
    np.cumsum(cnt_flat[:-1], out=group_start[1:])
    ranks[order] = np.arange(E) - group_start[key[order]]

    chunk_of = chunk_base[tl, erel, half] + ranks // 128              # [E]
    part_of = ranks % 128
    rd = col % 128
    idx_all[core, tl, part_of, chunk_of] = (row - half * NHALF).astype(np.int16)
    idx2_all[core, tl, part_of, chunk_of] = (erel * 128 + rd).astype(np.int16)
    rds_all[core, tl, part_of, chunk_of] = rd.astype(NPBF16)

    # chunk -> relation map per tile (lo section then hi section)
    chunk_rel = []
    for t in range(TPC):
        rels = []
        for hh in range(2):
            for r in range(R):
                rels += [r] * int(n_chunks[t, r, hh])
        chunk_rel.append(rels)

    # int16 gather index lists, wrap16 layout, concatenated per core:
    # per tile: [kv-lo (C_lo*8) | kv-hi (C_hi*8)] columns
    idx16 = np.zeros((NCORES, 128, 8 * int(C_t.sum())), np.int16)
    seg_off = []          # per tile: (lo_off, hi_off) in i16 columns
    off = 0
    for t in range(TPC):
        clo, chi, ct = int(C_lo_t[t]), int(C_hi_t[t]), int(C_t[t])
        seg_off.append((off, off + clo * 8))
        for c in range(NCORES):
            g = idx_all[c, t, :, :ct]          # [128, C]
            lo = _wrap16(np.ascontiguousarray(g[:, :clo].T).reshape(-1))
            hi = _wrap16(np.ascontiguousarray(g[:, clo:ct].T).reshape(-1))
            idx16[c, :, off:off + clo * 8] = lo
            idx16[c, :, off + clo * 8:off + ct * 8] = hi
        off += ct * 8
    idx16 = np.ascontiguousarray(idx16[:, :, :off])
    TOT16 = off

    # one-hot O[e, (c, j)] and O^T[j, (c, e)] shipped directly as fp8
    # (0/1 exact); padded slots (rds=999) give all-zero rows/cols
    jj = np.arange(128)
    o_all = np.zeros((NCORES, TPC, 128, Cmax * 128), NPF8)
    ot_all = np.zeros((NCORES, TPC, 128, Cmax * 128), NPF8)
    rds_f = rds_all.astype(np.float32)
    for c in range(NCORES):
        for t in range(TPC):
            oh = (rds_f[c, t][:, :, None] == jj[None, None, :])   # [e, C, j]
            o_all[c, t] = oh.reshape(128, -1).astype(NPF8)
            ot_all[c, t] = oh.transpose(2, 1, 0).reshape(128, -1).astype(NPF8)

    hT = np.ascontiguousarray(np.asarray(h, np.float32).T.astype(NPBF16))  # [128, N]
    hexp = np.zeros((NH, D), np.float32)      # head expander: hexp[h, d] = (d//DK == h)
    for hh in range(NH):
        hexp[hh, hh * DK:(hh + 1) * DK] = 1.0


    in_maps = []
    for c in range(NCORES):
        t_c = (c * NPC) // NPT
        in_maps.append({
            "ht": hT,
            "ownht": np.ascontiguousarray(hT[:, c * NPC:(c + 1) * NPC]),
            "wkv": np.ascontiguousarray(
                WKV.transpose(1, 0, 2).reshape(D, T * 256).astype(NPBF16)),
            "wqa": np.ascontiguousarray(
                WQA[t_c].transpose(1, 0, 2).reshape(D, R * D).astype(NPBF16)),
            "wmo": np.ascontiguousarray(
                WMO[:, t_c].transpose(1, 0, 2).reshape(D, R * D).astype(NPBF16)),
            "idx16": idx16[c],
            "oall": o_all[c],
            "otall": ot_all[c],
            "hexp": hexp,
        })
    meta = dict(chunk_rel=chunk_rel, C_lo=C_lo_t, C_hi=C_hi_t, C_t=C_t,
                Cmax=Cmax, seg_off=seg_off, TOT16=TOT16)
    return in_maps, meta


def _build_program(meta):
    chunk_rel, C_lo, C_hi, C_t = (meta["chunk_rel"], meta["C_lo"],
                                  meta["C_hi"], meta["C_t"])
    Cmax, seg_off, TOT16 = meta["Cmax"], meta["seg_off"], meta["TOT16"]

    nc = bacc_mod.Bacc(num_swdge_queues=4)
    ht_ext = nc.declare_dram_parameter("ht", [D, N], BF16, isOutput=False)
    ownht_ext = nc.declare_dram_parameter("ownht", [D, NPC], BF16, isOutput=False)
    wkv_ext = nc.declare_dram_parameter("wkv", [D, T * 256], BF16, isOutput=False)
    wqa_ext = nc.declare_dram_parameter("wqa", [D, R * D], BF16, isOutput=False)
    wmo_ext = nc.declare_dram_parameter("wmo", [D, R * D], BF16, isOutput=False)
    idx16_ext = nc.declare_dram_parameter("idx16", [128, TOT16], I16, isOutput=False)
    oall_ext = nc.declare_dram_parameter("oall", [TPC, 128, Cmax * 128], F8, isOutput=False)
    otall_ext = nc.declare_dram_parameter("otall", [TPC, 128, Cmax * 128], F8, isOutput=False)
    hexp_ext = nc.declare_dram_parameter("hexp", [NH, D], F32, isOutput=False)
    out_ext = nc.declare_dram_parameter("out", [NPC, D], F32, isOutput=True)

    kv_dram = nc.dram_tensor("kv_table", [N, 2 * D], BF16)

    NB = TT // 8               # phase-1 batches of 8 tiles (1024 nodes)
    Exp = mybir.ActivationFunctionType.Exp
    Copy = mybir.ActivationFunctionType.Copy

    with tile_mod.TileContext(nc) as tc:
        with (
            tc.tile_pool(name="const", bufs=1) as cp,
            tc.tile_pool(name="ph1", bufs=3) as p1,
            tc.tile_pool(name="qatp", bufs=4) as qp,
            tc.tile_pool(name="tile", bufs=4) as tp,
            tc.tile_pool(name="ps512", bufs=3, space="PSUM") as ps512,
            tc.tile_pool(name="ps_at", bufs=2, space="PSUM") as ps_at,
            tc.tile_pool(name="ps_sm", bufs=1, space="PSUM") as ps_sm,
        ):
            hexp_sb = cp.tile([NH, D], F32)
            nc.sync.dma_start(out=hexp_sb[:], in_=hexp_ext[:])
            ident = cp.tile([128, 128], F32)
            make_identity(nc, ident[:])
            wkv_sb = cp.tile([128, T * 256], BF16)
            nc.sync.dma_start(out=wkv_sb[:], in_=wkv_ext[:])
            wqa_sb = cp.tile([128, R * D], BF16)
            nc.sync.dma_start(out=wqa_sb[:], in_=wqa_ext[:])
            wmo_sb = cp.tile([128, R * D], BF16)
            nc.sync.dma_start(out=wmo_sb[:], in_=wmo_ext[:])
            ownht = cp.tile([128, NPC], BF16)
            nc.sync.dma_start(out=ownht[:], in_=ownht_ext[:])
            idx16_sb = cp.tile([128, TOT16], I16)
            nc.sync.dma_start(out=idx16_sb[:], in_=idx16_ext[:])

            # ---- phase 1: bf16 k|v table for all N nodes ----
            for b in range(NB):
                htc = p1.tile([128, 1024], BF16, tag="htc")
                nc.sync.dma_start(out=htc[:], in_=ht_ext[:, b * 1024:(b + 1) * 1024])
                kvs = p1.tile([128, 2048], BF16, tag="kvs")
                for i in range(4):
                    ty = (8 * b + 2 * i) // (NPT // 128)
                    kvp = ps512.tile([128, 512], F32, tag="p512")
                    for j in range(2):
                        nc.tensor.matmul(kvp[:, j * 256:(j + 1) * 256],
                                         lhsT=htc[:, (2 * i + j) * 128:(2 * i + j + 1) * 128],
                                         rhs=wkv_sb[:, ty * 256:(ty + 1) * 256],
                                         start=True, stop=True)
                    if i % 2 == 0:
                        nc.scalar.activation(out=kvs[:, i * 512:(i + 1) * 512],
                                             in_=kvp[:], func=Copy)
                    else:
                        nc.vector.tensor_copy(kvs[:, i * 512:(i + 1) * 512], kvp[:])
                nc.sync.dma_start(
                    out=kv_dram[b * 1024:(b + 1) * 1024, :]
                        .rearrange("(t p) k -> p t k", p=128),
                    in_=kvs[:].rearrange("p (t k) -> p t k", t=8))

            # ---- phase 2: per node-tile edge processing ----
            qn = 0
            for tl in range(TPC):
                C = int(C_t[tl])
                CL = int(C_lo[tl])
                CH = int(C_hi[tl])
                rels = chunk_rel[tl]
                lo_off, hi_off = seg_off[tl]

                # per-tile rotated queries (stay in SBUF)
                qat = qp.tile([128, R * D], BF16, tag="qat")
                for i in range(2):
                    qah = ps512.tile([128, 512], F32, tag="p512")
                    nc.tensor.matmul(qah[:],
                                     lhsT=ownht[:, tl * 128:(tl + 1) * 128],
                                     rhs=wqa_sb[:, i * 512:(i + 1) * 512],
                                     start=True, stop=True)
                    nc.scalar.activation(out=qat[:, i * 512:(i + 1) * 512],
                                         in_=qah[:], func=Copy)

                # single_packet dma_gather caps at 64 descs/lane = 1024 idxs
                # = 8 chunks per op; split larger gathers into 8-chunk spans
                def gather_spans(dst, dst_col0, src_ap, idx_col0, nch, es):
                    nonlocal qn
                    for s0 in range(0, nch, 8):
                        sc = min(8, nch - s0)
                        nc.gpsimd.dma_gather(
                            dst[:, (dst_col0 + s0) * es:(dst_col0 + s0 + sc) * es]
                                .rearrange("p (c x) -> p c x", x=es),
                            src_ap,
                            idx16_sb[:, idx_col0 + s0 * 8:idx_col0 + (s0 + sc) * 8],
                            sc * 128, sc * 128, es, queue_num=qn % 4)
                        qn += 1

                kvg = tp.tile([128, Cmax * 256], BF16, tag="kvg")
                if CL:
                    gather_spans(kvg, 0, kv_dram[0:NHALF, :], lo_off, CL, 256)
                if CH:
                    gather_spans(kvg, CL, kv_dram[NHALF:N, :], hi_off, CH, 256)

                # one-hot O[e, (c,j)] and O^T[j, (c,e)] DMAd from host (fp8)
                Oall = tp.tile([128, Cmax * 128], F8, tag="Oall")
                nc.sync.dma_start(out=Oall[:, :C * 128], in_=oall_ext[tl, :, :C * 128])
                OT = tp.tile([128, Cmax * 128], F8, tag="OT")
                nc.sync.dma_start(out=OT[:, :C * 128], in_=otall_ext[tl, :, :C * 128])

                # qep[e, d] = qat[dst_e, rel_e, d] via one-hot matmuls, in
                # PSUM waves of 8 chunks; then attn = sum_d qep * k per head
                prod = tp.tile([128, Cmax * 128], BF16, tag="prod")
                for w0 in range(0, C, 4):
                    nw = min(4, C - w0)
                    qepw = ps512.tile([128, 512], F32, tag="p512")
                    for c in range(w0, w0 + nw):
                        rc = rels[c]
                        nc.tensor.matmul(qepw[:, (c - w0) * 128:(c - w0 + 1) * 128],
                                         lhsT=OT[:, c * 128:(c + 1) * 128],
                                         rhs=qat[:, rc * 128:(rc + 1) * 128],
                                         start=True, stop=True)
                    nc.vector.tensor_tensor(
                        out=prod[:, w0 * 128:(w0 + nw) * 128]
                            .rearrange("p (c d) -> p c d", c=nw),
                        in0=qepw[:, :nw * 128].rearrange("p (c d) -> p c d", c=nw),
                        in1=kvg[:, :C * 256].rearrange("p (c x) -> p c x", c=C)[:, w0:w0 + nw, 0:128],
                        op=mybir.AluOpType.mult,
                    )

                # pairwise tree-reduce of the 32 dk dims per head
                cur, n = prod, C * 128
                for lvl in range(4):
                    nxt = tp.tile([128, Cmax * (64 >> lvl)], BF16, tag=f"red{lvl}")
                    v = cur[:, :n].rearrange("p (g t) -> p g t", t=2)
                    nc.vector.tensor_tensor(
                        out=nxt[:, :n // 2].rearrange("p (g t) -> p g t", t=1),
                        in0=v[:, :, 0:1], in1=v[:, :, 1:2],
                        op=mybir.AluOpType.add,
                    )
                    cur, n = nxt, n // 2
                attn = tp.tile([128, Cmax * NH], F32, tag="attn")
                v = cur[:, :n].rearrange("p (g t) -> p g t", t=2)
                nc.vector.tensor_tensor(
                    out=attn[:, :C * NH].rearrange("p (g t) -> p g t", t=1),
                    in0=v[:, :, 0:1], in1=v[:, :, 1:2],
                    op=mybir.AluOpType.add,
                )
                wv = tp.tile([128, Cmax * NH], BF16, tag="wv")
                nc.scalar.activation(out=wv[:, :C * NH], in_=attn[:, :C * NH], func=Exp)

                # wm[e, d] = w[e, h(d)] * v_raw[src_e, d]
                wmt = tp.tile([128, Cmax * 128], BF16, tag="wmt")
                nc.vector.tensor_tensor(
                    out=wmt[:, :C * 128].rearrange("p (c h d) -> p c h d", c=C, h=NH),
                    in0=kvg[:, :C * 256].rearrange("p (c x) -> p c x", c=C)[:, :, 128:256]
                        .rearrange("p c (h d) -> p c h d", h=NH),
                    in1=wv[:, :C * NH].rearrange("p (c h u) -> p c h u", c=C, u=1)
                        .to_broadcast([128, C, NH, DK]),
                    op=mybir.AluOpType.mult,
                )

                # segment sums into PSUM: A_T[d, j] per relation block + s[j, h]
                # PSUM start=True marks the whole 2KB zero region pending --
                # accumulation groups sharing a bank must run back-to-back,
                # so iterate chunks grouped by relation (data layout unchanged)
                ATp = ps_at.tile([128, R * D], F32, tag="ATp")
                sp = ps_sm.tile([128, 128], F32, tag="sm")
                order = sorted(range(C), key=lambda c: rels[c])
                for k, c in enumerate(order):
                    rc = rels[c]
                    first = (k == 0) or rels[order[k - 1]] != rc
                    last = (k == C - 1) or rels[order[k + 1]] != rc
                    nc.tensor.matmul(ATp[:, rc * D:(rc + 1) * D],
                                     lhsT=wmt[:, c * 128:(c + 1) * 128],
                                     rhs=Oall[:, c * 128:(c + 1) * 128],
                                     start=first, stop=last, skip_group_check=True)
                for c in range(C):
                    nc.tensor.matmul(sp[:, :NH], lhsT=Oall[:, c * 128:(c + 1) * 128],
                                     rhs=wv[:, c * NH:(c + 1) * NH],
                                     start=(c == 0), stop=(c == C - 1),
                                     skip_group_check=True)

                rec = tp.tile([128, NH], F32, tag="rec")
                nc.vector.reciprocal(rec[:], sp[:, :NH])
                # rts[d, j] = rec[j, h(d)] via tiny transpose + K=4 matmul
                # against the constant head-expander hexp[h, d] = (h(d) == h)
                rtp = ps_sm.tile([128, 128], F32, tag="sm")
                nc.tensor.transpose(rtp[:NH, :], rec[:], ident[:])
                recT = tp.tile([NH, 128], F32, tag="recT")
                nc.scalar.activation(out=recT[:], in_=rtp[:NH, :], func=Copy)
                rts2 = ps_sm.tile([128, 128], F32, tag="sm")
                nc.tensor.matmul(rts2[:], lhsT=hexp_sb[:], rhs=recT[:],
                                 start=True, stop=True)
                rts = tp.tile([128, 128], F32, tag="rts")
                nc.scalar.activation(out=rts[:], in_=rts2[:], func=Copy)

                Anorm = tp.tile([128, R * D], BF16, tag="Anorm")
                nc.vector.tensor_tensor(
                    out=Anorm[:].rearrange("p (r j) -> p r j", r=R),
                    in0=ATp[:].rearrange("p (r j) -> p r j", r=R),
                    in1=rts[:].rearrange("p (u j) -> p u j", u=1).to_broadcast([128, R, 128]),
                    op=mybir.AluOpType.mult,
                )

                outp = ps_sm.tile([128, 128], F32, tag="sm")
                for r in range(R):
                    nc.tensor.matmul(outp[:], lhsT=Anorm[:, r * D:(r + 1) * D],
                                     rhs=wmo_sb[:, r * D:(r + 1) * D],
                                     start=(r == 0), stop=(r == R - 1))
                osb = tp.tile([128, 128], F32, tag="osb")
                nc.scalar.activation(out=osb[:], in_=outp[:], func=Copy)
                nc.sync.dma_start(out=out_ext[tl * 128:(tl + 1) * 128, :], in_=osb[:])
    nc.compile()
    return nc


def kernel(h, k_linears, q_linears, v_linears, a_linears,
           relation_att, relation_msg, relation_pri, skip,
           row_idx, col_idx, eids, **_unused):
    in_maps, meta = _host_prep(
        h, k_linears, q_linears, v_linears, a_linears,
        relation_att, relation_msg, relation_pri, skip, row_idx, col_idx)
    nc = _build_program(meta)
    kw = {}
    if os.environ.get("KBENCH_TRACE"):
        kw = dict(trace=True, tmpdir=os.environ.get("KBENCH_TMPDIR") or None)
    res = run_bass_kernel_spmd(nc, in_maps, list(range(NCORES)), **kw)
    global LAST_RESULTS
    LAST_RESULTS = res
    out = np.concatenate([res.results[c]["out"] for c in range(NCORES)], axis=0)
    return out.astype(np.float32)


LAST_RESULTS = None


# revision 46
# speedup vs baseline: 1.3211x; 1.0745x over previous
"""HGT (heterogeneous graph transformer) layer on 8 trn2 NeuronCores.

Strategy (dst-node 1D sharding, uniform SPMD program):
  - Host folds all small weights:
      WKV[t]    = [W_k[t] | W_v[t]]                      (node-type projections)
      WQA[t,r]  = W_q[t] @ blockdiag(W_att[r] * pri[r,h]/sqrt(dk))
      WMO[r,t]  = blockdiag(W_msg[r]) @ (sigmoid(skip[t])*W_a[t])
    so the per-edge computation needs only RAW k/v of the src node:
      attn[e,h] = qat[rel][dst] . k_raw[src]     (per head, pri pre-folded)
      agg[j]    = sum_r (sum_{e in rel r, dst=j} w_e * v_raw[src]) @ WMO[r]
      out[j]    = agg[j] / s[j]                  (softmax denominator)
  - Each core owns a contiguous range of 6400 dst nodes (single node type).
    Per core the edges are grouped into (node-tile of 128 dst, relation,
    src-half, chunk of 128 edges); chunk structure is the max over cores so
    the SPMD program is identical on all cores, with per-core data padded.
    The src-half split (src < 25600 vs >=) keeps gather indices within
    int16 range for the batched SWDGE dma_gather instruction.
  - All matmul inputs are bf16 (4x PE rate vs fp32); PSUM accumulates fp32.
    The one-hot (edge,dst) matrices O and O^T are precomputed on host and
    shipped as fp8 (0/1 exact).
  - Host pre-transposes h to hT [128, N] bf16, so no PE transposes for the
    projections.  Phase 1 builds the bf16 [N,256] k|v table with batched
    1024-node DMAs; per tile, the rotated queries qat stay in SBUF.
  - Per node-tile, batched dma_gather ops (<=1024 indices each, the
    single-packet 64-descs/lane cap) fetch all edges' k|v rows, spread
    round-robin over the 4 SWDGE queues; a monkeypatched DMASW-lane
    assignment pins each queue to its own semaphore lanes.
  - Per-edge q is gathered on the PE: qep = O^T @ qat into PSUM waves;
    attn = pairwise tree-reduce of qep*k (grouped TENSOR_REDUCE is slow).
  - Segment sums over dst are one-hot matmuls into PSUM accumulated per
    relation; PSUM start=True marks the whole 2KB zero region pending, so
    accumulation groups sharing a bank are iterated back-to-back per rel.
  - The softmax denominator reciprocal is broadcast dk-wise via a tiny
    transpose + K=4 matmul against a constant head-expander matrix.
  - Padded edge slots have all-zero one-hot rows -> zero contribution to
    both numerator and denominator; their gathered values are real table
    rows so no NaN risk.
  - Softmax skips the segment-max subtraction: scores are O(1) here, and
    exp(s)/sum(exp(s)) is invariant to the shift.
"""

import os
import sys

sys.path.insert(0, "/opt/trn_rl_repo")

import ml_dtypes
import numpy as np

import concourse.bass as bass
import concourse.bacc as bacc_mod
import concourse.mybir as mybir
import concourse.tile as tile_mod
import concourse.tile_sem_assignment as _tsa
from concourse.bass_utils import run_bass_kernel_spmd
from concourse.masks import make_identity

# The tile framework rotates SWDGE DMAs over the 8 DMASW semaphore lanes in
# scheduled order, but each physical semaphore is locked to one SWDGE queue.
# To run gathers on all 4 queues concurrently, pin each queue to its own pair
# of lanes so a semaphore only ever sees one queue.
_ORIG_ASSIGN_TICK = _tsa.TileClockTick._assign_tick


def _qlane_assign_tick(self, inst):
    qnum = getattr(inst, "queue_num", None)
    if (qnum is not None and inst.engine == mybir.EngineType.Pool
            and isinstance(inst, _tsa.DMAInst)
            and not isinstance(inst, _tsa.bass_isa.UserSyncedRemoteDMADescs)
            and self.swdge_sem_count >= 8):
        cnt = getattr(self, "_qlane_cnt", None)
        if cnt is None:
            cnt = self._qlane_cnt = [0, 0, 0, 0]
        lanes = self.swdge_sem_count // 4
        save = self.next_sw_dma_idx
        self.next_sw_dma_idx = qnum * lanes + (cnt[qnum] % lanes)
        cnt[qnum] += 1
        try:
            return _ORIG_ASSIGN_TICK(self, inst)
        finally:
            self.next_sw_dma_idx = save
    return _ORIG_ASSIGN_TICK(self, inst)


_tsa.TileClockTick._assign_tick = _qlane_assign_tick

F32 = mybir.dt.float32
BF16 = mybir.dt.bfloat16
F8 = mybir.dt.float8e4
I16 = mybir.dt.int16
NPBF16 = ml_dtypes.bfloat16
NPF8 = ml_dtypes.float8_e4m3

N, E, T, R, NH, DK, D = 51200, 640000, 4, 8, 4, 32, 128
NCORES = 8
NPC = N // NCORES          # 6400 nodes per core
TPC = NPC // 128           # 50 node-tiles per core
TT = N // 128              # 400 table tiles
NPT = N // T               # nodes per type
EPR = E // R               # edges per relation
NHALF = N // 2             # src-half split for int16 gather indices
SQRT_DK = float(np.sqrt(DK))


def _blockdiag(W):
    """[R,H,dk,dk] -> [R,D,D] block-diagonal per head."""
    out = np.zeros((R, D, D), np.float32)
    for r in range(R):
        for hh in range(NH):
            out[r, hh * DK:(hh + 1) * DK, hh * DK:(hh + 1) * DK] = W[r, hh]
    return out


def _wrap16(L):
    """Linear int16 index list -> [128, ceil(n/16)] SBUF layout (wrapped in 16
    partitions, replicated to all 8 16-partition groups)."""
    n = L.shape[0]
    F = -(-n // 16)
    pad = np.zeros(F * 16, np.int16)
    pad[:n] = L
    seg = pad.reshape(F, 16).T               # [16, F]
    return np.tile(seg, (8, 1))              # [128, F]


def _host_prep(h, k_linears, q_linears, v_linears, a_linears,
               relation_att, relation_msg, relation_pri, skip,
               row_idx, col_idx):
    pri = np.asarray(relation_pri, np.float32) / SQRT_DK               # [R,H]
    att = np.asarray(relation_att, np.float32) * pri[:, :, None, None]
    Watt = _blockdiag(att)
    Wmsg = _blockdiag(np.asarray(relation_msg, np.float32))
    skip = np.asarray(skip, np.float32)
    Wout = (1.0 / (1.0 + np.exp(-skip))).astype(np.float32) * np.asarray(a_linears, np.float32)
    WQA = np.einsum("tab,rbc->trac", np.asarray(q_linears, np.float32), Watt)
    WMO = np.einsum("rab,tbc->rtac", Wmsg, Wout)
    WKV = np.concatenate([np.asarray(k_linears, np.float32),
                          np.asarray(v_linears, np.float32)], axis=2)  # [T,D,256]

    row = np.asarray(row_idx, np.int64)
    col = np.asarray(col_idx, np.int64)
    erel = np.arange(E, dtype=np.int64) // EPR
    half = (row >= NHALF).astype(np.int64)

    core = col // NPC
    tl = (col % NPC) // 128
    # per-(core,tile,rel,half) edge counts
    key = ((core * TPC + tl) * R + erel) * 2 + half
    counts = np.bincount(key, minlength=NCORES * TPC * R * 2).reshape(NCORES, TPC, R, 2)
    maxcnt = counts.max(axis=0)                                       # [TPC,R,2]
    n_chunks = -(-maxcnt // 128)                                      # ceil
    # ensure at least one chunk per tile (degenerate safety)
    C_lo_t = n_chunks[:, :, 0].sum(axis=1)
    C_hi_t = n_chunks[:, :, 1].sum(axis=1)
    # chunk index base per (tile, rel, half): lo chunks first, then hi
    chunk_base = np.zeros((TPC, R, 2), np.int64)
    for t in range(TPC):
        off = 0
        for r in range(R):
            chunk_base[t, r, 0] = off
            off += n_chunks[t, r, 0]
        for r in range(R):
            chunk_base[t, r, 1] = off
            off += n_chunks[t, r, 1]
    C_t = C_lo_t + C_hi_t
    Cmax = int(C_t.max())

    # per-core padded metadata arrays
    idx_all = np.zeros((NCORES, TPC, 128, Cmax), np.int16)   # kv idx (half-local)
    idx2_all = np.zeros((NCORES, TPC, 128, Cmax), np.int16)  # qat idx (tile-local)
    rds_all = np.full((NCORES, TPC, 128, Cmax), 999.0, NPBF16)

    order = np.argsort(key, kind="stable")
    ranks = np.empty(E, np.int64)
    group_start = np.zeros(NCORES * TPC * R * 2, np.int64)
    cnt_flat = counts.reshape(-1)
    np.cumsum(cnt_flat[:-1], out=group_start[1:])
    ranks[order] = np.arange(E) - group_start[key[order]]

    chunk_of = chunk_base[tl, erel, half] + ranks // 128              # [E]
    part_of = ranks % 128
    rd = col % 128
    idx_all[core, tl, part_of, chunk_of] = (row - half * NHALF).astype(np.int16)
    idx2_all[core, tl, part_of, chunk_of] = (erel * 128 + rd).astype(np.int16)
    rds_all[core, tl, part_of, chunk_of] = rd.astype(NPBF16)

    # chunk -> relation map per tile (lo section then hi section)
    chunk_rel = []
    for t in range(TPC):
        rels = []
        for hh in range(2):
            for r in range(R):
                rels += [r] * int(n_chunks[t, r, hh])
        chunk_rel.append(rels)

    # int16 gather index lists, wrap16 layout, concatenated per core:
    # per tile: [kv-lo (C_lo*8) | kv-hi (C_hi*8)] columns
    idx16 = np.zeros((NCORES, 128, 8 * int(C_t.sum())), np.int16)
    seg_off = []          # per tile: (lo_off, hi_off) in i16 columns
    off = 0
    for t in range(TPC):
        clo, chi, ct = int(C_lo_t[t]), int(C_hi_t[t]), int(C_t[t])
        seg_off.append((off, off + clo * 8))
        for c in range(NCORES):
            g = idx_all[c, t, :, :ct]          # [128, C]
            lo = _wrap16(np.ascontiguousarray(g[:, :clo].T).reshape(-1))
            hi = _wrap16(np.ascontiguousarray(g[:, clo:ct].T).reshape(-1))
            idx16[c, :, off:off + clo * 8] = lo
            idx16[c, :, off + clo * 8:off + ct * 8] = hi
        off += ct * 8
    idx16 = np.ascontiguousarray(idx16[:, :, :off])
    TOT16 = off

    # one-hot O[e, (c, j)] and O^T[j, (c, e)] shipped directly as fp8
    # (0/1 exact); padded slots (rds=999) give all-zero rows/cols
    jj = np.arange(128)
    o_all = np.zeros((NCORES, TPC, 128, Cmax * 128), NPF8)
    ot_all = np.zeros((NCORES, TPC, 128, Cmax * 128), NPF8)
    rds_f = rds_all.astype(np.float32)
    for c in range(NCORES):
        for t in range(TPC):
            oh = (rds_f[c, t][:, :, None] == jj[None, None, :])   # [e, C, j]
            o_all[c, t] = oh.reshape(128, -1).astype(NPF8)
            ot_all[c, t] = oh.transpose(2, 1, 0).reshape(128, -1).astype(NPF8)

    hT = np.ascontiguousarray(np.asarray(h, np.float32).T.astype(NPBF16))  # [128, N]
    hexp = np.zeros((NH, D), np.float32)      # head expander: hexp[h, d] = (d//DK == h)
    for hh in range(NH):
        hexp[hh, hh * DK:(hh + 1) * DK] = 1.0


    in_maps = []
    for c in range(NCORES):
        t_c = (c * NPC) // NPT
        in_maps.append({
            "ht": hT,
            "ownht": np.ascontiguousarray(hT[:, c * NPC:(c + 1) * NPC]),
            "wkv": np.ascontiguousarray(
                WKV.transpose(1, 0, 2).reshape(D, T * 256).astype(NPBF16)),
            "wqa": np.ascontiguousarray(
                WQA[t_c].transpose(1, 0, 2).reshape(D, R * D).astype(NPBF16)),
            "wmo": np.ascontiguousarray(
                WMO[:, t_c].transpose(1, 0, 2).reshape(D, R * D).astype(NPBF16)),
            "idx16": idx16[c],
            "oall": o_all[c],
            "otall": ot_all[c],
            "hexp": hexp,
        })
    meta = dict(chunk_rel=chunk_rel, C_lo=C_lo_t, C_hi=C_hi_t, C_t=C_t,
                Cmax=Cmax, seg_off=seg_off, TOT16=TOT16)
    return in_maps, meta


def _build_program(meta):
    chunk_rel, C_lo, C_hi, C_t = (meta["chunk_rel"], meta["C_lo"],
                                  meta["C_hi"], meta["C_t"])
    Cmax, seg_off, TOT16 = meta["Cmax"], meta["seg_off"], meta["TOT16"]

    nc = bacc_mod.Bacc(num_swdge_queues=4)
    ht_ext = nc.declare_dram_parameter("ht", [D, N], BF16, isOutput=False)
    ownht_ext = nc.declare_dram_parameter("ownht", [D, NPC], BF16, isOutput=False)
    wkv_ext = nc.declare_dram_parameter("wkv", [D, T * 256], BF16, isOutput=False)
    wqa_ext = nc.declare_dram_parameter("wqa", [D, R * D], BF16, isOutput=False)
    wmo_ext = nc.declare_dram_parameter("wmo", [D, R * D], BF16, isOutput=False)
    idx16_ext = nc.declare_dram_parameter("idx16", [128, TOT16], I16, isOutput=False)
    oall_ext = nc.declare_dram_parameter("oall", [TPC, 128, Cmax * 128], F8, isOutput=False)
    otall_ext = nc.declare_dram_parameter("otall", [TPC, 128, Cmax * 128], F8, isOutput=False)
    hexp_ext = nc.declare_dram_parameter("hexp", [NH, D], F32, isOutput=False)
    out_ext = nc.declare_dram_parameter("out", [NPC, D], F32, isOutput=True)

    kv_dram = nc.dram_tensor("kv_table", [N, 2 * D], BF16)

    NB = TT // 8               # phase-1 batches of 8 tiles (1024 nodes)
    Exp = mybir.ActivationFunctionType.Exp
    Copy = mybir.ActivationFunctionType.Copy

    with tile_mod.TileContext(nc) as tc:
        with (
            tc.tile_pool(name="const", bufs=1) as cp,
            tc.tile_pool(name="ph1", bufs=3) as p1,
            tc.tile_pool(name="qatp", bufs=4) as qp,
            tc.tile_pool(name="tile", bufs=4) as tp,
            tc.tile_pool(name="ps512", bufs=3, space="PSUM") as ps512,
            tc.tile_pool(name="ps_at", bufs=2, space="PSUM") as ps_at,
            tc.tile_pool(name="ps_sm", bufs=1, space="PSUM") as ps_sm,
        ):
            hexp_sb = cp.tile([NH, D], F32)
            nc.sync.dma_start(out=hexp_sb[:], in_=hexp_ext[:])
            ident = cp.tile([128, 128], F32)
            make_identity(nc, ident[:])
            wkv_sb = cp.tile([128, T * 256], BF16)
            nc.sync.dma_start(out=wkv_sb[:], in_=wkv_ext[:])
            wqa_sb = cp.tile([128, R * D], BF16)
            nc.sync.dma_start(out=wqa_sb[:], in_=wqa_ext[:])
            wmo_sb = cp.tile([128, R * D], BF16)
            nc.sync.dma_start(out=wmo_sb[:], in_=wmo_ext[:])
            ownht = cp.tile([128, NPC], BF16)
            nc.sync.dma_start(out=ownht[:], in_=ownht_ext[:])
            idx16_sb = cp.tile([128, TOT16], I16)
            nc.sync.dma_start(out=idx16_sb[:], in_=idx16_ext[:])

            # ---- phase 1: bf16 k|v table for all N nodes ----
            for b in range(NB):
                htc = p1.tile([128, 1024], BF16, tag="htc")
                nc.sync.dma_start(out=htc[:], in_=ht_ext[:, b * 1024:(b + 1) * 1024])
                kvs = p1.tile([128, 2048], BF16, tag="kvs")
                for i in range(4):
                    ty = (8 * b + 2 * i) // (NPT // 128)
                    kvp = ps512.tile([128, 512], F32, tag="p512")
                    for j in range(2):
                        nc.tensor.matmul(kvp[:, j * 256:(j + 1) * 256],
                                         lhsT=htc[:, (2 * i + j) * 128:(2 * i + j + 1) * 128],
                                         rhs=wkv_sb[:, ty * 256:(ty + 1) * 256],
                                         start=True, stop=True)
                    if i % 2 == 0:
                        nc.scalar.activation(out=kvs[:, i * 512:(i + 1) * 512],
                                             in_=kvp[:], func=Copy)
                    else:
                        nc.vector.tensor_copy(kvs[:, i * 512:(i + 1) * 512], kvp[:])
                nc.sync.dma_start(
                    out=kv_dram[b * 1024:(b + 1) * 1024, :]
                        .rearrange("(t p) k -> p t k", p=128),
                    in_=kvs[:].rearrange("p (t k) -> p t k", t=8))

            # ---- phase 2: per node-tile edge processing ----
            qn = 0
            for tl in range(TPC):
                C = int(C_t[tl])
                CL = int(C_lo[tl])
                CH = int(C_hi[tl])
                rels = chunk_rel[tl]
                lo_off, hi_off = seg_off[tl]

                # per-tile rotated queries (stay in SBUF)
                qat = qp.tile([128, R * D], BF16, tag="qat")
                for i in range(2):
                    qah = ps512.tile([128, 512], F32, tag="p512")
                    nc.tensor.matmul(qah[:],
                                     lhsT=ownht[:, tl * 128:(tl + 1) * 128],
                                     rhs=wqa_sb[:, i * 512:(i + 1) * 512],
                                     start=True, stop=True)
                    nc.scalar.activation(out=qat[:, i * 512:(i + 1) * 512],
                                         in_=qah[:], func=Copy)

                # single_packet dma_gather caps at 64 descs/lane = 1024 idxs
                # = 8 chunks per op; split larger gathers into 8-chunk spans
                def gather_spans(dst, dst_col0, src_ap, idx_col0, nch, es):
                    nonlocal qn
                    for s0 in range(0, nch, 8):
                        sc = min(8, nch - s0)
                        nc.gpsimd.dma_gather(
                            dst[:, (dst_col0 + s0) * es:(dst_col0 + s0 + sc) * es]
                                .rearrange("p (c x) -> p c x", x=es),
                            src_ap,
                            idx16_sb[:, idx_col0 + s0 * 8:idx_col0 + (s0 + sc) * 8],
                            sc * 128, sc * 128, es, queue_num=qn % 4)
                        qn += 1

                kvg = tp.tile([128, Cmax * 256], BF16, tag="kvg")
                if CL:
                    gather_spans(kvg, 0, kv_dram[0:NHALF, :], lo_off, CL, 256)
                if CH:
                    gather_spans(kvg, CL, kv_dram[NHALF:N, :], hi_off, CH, 256)

                # one-hot O[e, (c,j)] and O^T[j, (c,e)] DMAd from host (fp8)
                Oall = tp.tile([128, Cmax * 128], F8, tag="Oall")
                nc.sync.dma_start(out=Oall[:, :C * 128], in_=oall_ext[tl, :, :C * 128])
                OT = tp.tile([128, Cmax * 128], F8, tag="OT")
                nc.sync.dma_start(out=OT[:, :C * 128], in_=otall_ext[tl, :, :C * 128])

                # qep[e, d] = qat[dst_e, rel_e, d] via one-hot matmuls, in
                # PSUM waves of 8 chunks; then attn = sum_d qep * k per head
                prod = tp.tile([128, Cmax * 128], BF16, tag="prod")
                for w0 in range(0, C, 4):
                    nw = min(4, C - w0)
                    qepw = ps512.tile([128, 512], F32, tag="p512")
                    for c in range(w0, w0 + nw):
                        rc = rels[c]
                        nc.tensor.matmul(qepw[:, (c - w0) * 128:(c - w0 + 1) * 128],
                                         lhsT=OT[:, c * 128:(c + 1) * 128],
                                         rhs=qat[:, rc * 128:(rc + 1) * 128],
                                         start=True, stop=True)
                    nc.vector.tensor_tensor(
                        out=prod[:, w0 * 128:(w0 + nw) * 128]
                            .rearrange("p (c d) -> p c d", c=nw),
                        in0=qepw[:, :nw * 128].rearrange("p (c d) -> p c d", c=nw),
                        in1=kvg[:, :C * 256].rearrange("p (c x) -> p c x", c=C)[:, w0:w0 + nw, 0:128],
                        op=mybir.AluOpType.mult,
                    )

                # pairwise tree-reduce of the 32 dk dims per head
                cur, n = prod, C * 128
                for lvl in range(4):
                    nxt = tp.tile([128, Cmax * (64 >> lvl)], BF16, tag=f"red{lvl}")
                    v = cur[:, :n].rearrange("p (g t) -> p g t", t=2)
                    nc.vector.tensor_tensor(
                        out=nxt[:, :n // 2].rearrange("p (g t) -> p g t", t=1),
                        in0=v[:, :, 0:1], in1=v[:, :, 1:2],
                        op=mybir.AluOpType.add,
                    )
                    cur, n = nxt, n // 2
                attn = tp.tile([128, Cmax * NH], F32, tag="attn")
                v = cur[:, :n].rearrange("p (g t) -> p g t", t=2)
                nc.vector.tensor_tensor(
                    out=attn[:, :C * NH].rearrange("p (g t) -> p g t", t=1),
                    in0=v[:, :, 0:1], in1=v[:, :, 1:2],
                    op=mybir.AluOpType.add,
                )
                wv = tp.tile([128, Cmax * NH], BF16, tag="wv")
                nc.scalar.activation(out=wv[:, :C * NH], in_=attn[:, :C * NH], func=Exp)

                # wm[e, d] = w[e, h(d)] * v_raw[src_e, d]
                wmt = tp.tile([128, Cmax * 128], BF16, tag="wmt")
                nc.vector.tensor_tensor(
                    out=wmt[:, :C * 128].rearrange("p (c h d) -> p c h d", c=C, h=NH),
                    in0=kvg[:, :C * 256].rearrange("p (c x) -> p c x", c=C)[:, :, 128:256]
                        .rearrange("p c (h d) -> p c h d", h=NH),
                    in1=wv[:, :C * NH].rearrange("p (c h u) -> p c h u", c=C, u=1)
                        .to_broadcast([128, C, NH, DK]),
                    op=mybir.AluOpType.mult,
                )

                # segment sums into PSUM: A_T[d, j] per relation block + s[j, h]
                # PSUM start=True marks the whole 2KB zero region pending --
                # accumulation groups sharing a bank must run back-to-back,
                # so iterate chunks grouped by relation (data layout unchanged)
                ATp = ps_at.tile([128, R * D], F32, tag="ATp")
                sp = ps_sm.tile([128, 128], F32, tag="sm")
                order = sorted(range(C), key=lambda c: rels[c])
                for k, c in enumerate(order):
                    rc = rels[c]
                    first = (k == 0) or rels[order[k - 1]] != rc
                    last = (k == C - 1) or rels[order[k + 1]] != rc
                    nc.tensor.matmul(ATp[:, rc * D:(rc + 1) * D],
                                     lhsT=wmt[:, c * 128:(c + 1) * 128],
                                     rhs=Oall[:, c * 128:(c + 1) * 128],
                                     start=first, stop=last, skip_group_check=True)
                for c in range(C):
                    nc.tensor.matmul(sp[:, :NH], lhsT=Oall[:, c * 128:(c + 1) * 128],
                                     rhs=wv[:, c * NH:(c + 1) * NH],
                                     start=(c == 0), stop=(c == C - 1),
                                     skip_group_check=True)

                rec = tp.tile([128, NH], F32, tag="rec")
                nc.vector.reciprocal(rec[:], sp[:, :NH])
                # rts[d, j] = rec[j, h(d)] via tiny transpose + K=4 matmul
                # against the constant head-expander hexp[h, d] = (h(d) == h)
                rtp = ps_sm.tile([128, 128], F32, tag="sm")
                nc.tensor.transpose(rtp[:NH, :], rec[:], ident[:])
                recT = tp.tile([NH, 128], F32, tag="recT")
                nc.scalar.activation(out=recT[:], in_=rtp[:NH, :], func=Copy)
                rts2 = ps_sm.tile([128, 128], F32, tag="sm")
                nc.tensor.matmul(rts2[:], lhsT=hexp_sb[:], rhs=recT[:],
                                 start=True, stop=True)
                rts = tp.tile([128, 128], F32, tag="rts")
                nc.scalar.activation(out=rts[:], in_=rts2[:], func=Copy)

                Anorm = tp.tile([128, R * D], BF16, tag="Anorm")
                nc.vector.tensor_tensor(
                    out=Anorm[:].rearrange("p (r j) -> p r j", r=R),
                    in0=ATp[:].rearrange("p (r j) -> p r j", r=R),
                    in1=rts[:].rearrange("p (u j) -> p u j", u=1).to_broadcast([128, R, 128]),
                    op=mybir.AluOpType.mult,
                )

                outp = ps_sm.tile([128, 128], F32, tag="sm")
                for r in range(R):
                    nc.tensor.matmul(outp[:], lhsT=Anorm[:, r * D:(r + 1) * D],
                                     rhs=wmo_sb[:, r * D:(r + 1) * D],
                                     start=(r == 0), stop=(r == R - 1))
                osb = tp.tile([128, 128], F32, tag="osb")
                nc.scalar.activation(out=osb[:], in_=outp[:], func=Copy)
                nc.sync.dma_start(out=out_ext[tl * 128:(tl + 1) * 128, :], in_=osb[:])
    nc.compile()
    return nc


def kernel(h, k_linears, q_linears, v_linears, a_linears,
           relation_att, relation_msg, relation_pri, skip,
           row_idx, col_idx, eids, **_unused):
    in_maps, meta = _host_prep(
        h, k_linears, q_linears, v_linears, a_linears,
        relation_att, relation_msg, relation_pri, skip, row_idx, col_idx)
    nc = _build_program(meta)
    kw = {}
    if os.environ.get("KBENCH_TRACE"):
        kw = dict(trace=True, tmpdir=os.environ.get("KBENCH_TMPDIR") or None)
    res = run_bass_kernel_spmd(nc, in_maps, list(range(NCORES)), **kw)
    global LAST_RESULTS
    LAST_RESULTS = res
    out = np.concatenate([res.results[c]["out"] for c in range(NCORES)], axis=0)
    return out.astype(np.float32)


LAST_RESULTS = None
